# revision 5
# baseline (speedup 1.0000x reference)
"""Differential attention (nn_DifferentialAttention_84679575208071) on 8 TRN2
NeuronCores via Bass/Tile.

Sharding: hybrid data-parallel x tensor-parallel. Core c handles batch c//4 and
heads 4*(c%4) .. 4*(c%4)+4 (B=2, H=16 -> 2 batch groups x 4-way head split).
Each core computes its 4 heads' attention + group-norm + partial o_proj; the
host sums the 4 partial (S, D) outputs per batch (tensor-parallel o_proj
reduction) and stacks the 2 batches.

Per-core algorithm (all matmuls f32r = full-rate PE):
  xT = x^T (PE transposes)                        [D, S]
  qT/kT = (Wq/Wk slice)^T x^T with RoPE applied   [512, S] (2 branches x 4 heads x 64)
  v    = x Wv slice (row layout)                  [S, 256], augmented with a
         ones column per head -> PV matmul also yields softmax denominators
  attention, chunk-major: per (sq-chunk, head): both branches' score tiles go
         into one [128,1024] PSUM -> one wide exp (no max subtraction; scores
         are O(+-6) so fp32 exp is exact-safe) -> two PV matmuls [V|1]^T E^T
  diff = O1 - (lam*r1/r2) * O2  (group-norm is scale-invariant up to eps, so
         normalizing by r1 is folded in; eps is scaled by r1^2 to compensate)
  group-norm over each head's 64 channels, * gn_weight*(1-lam_init) + bias,
         then o_proj — both streamed per chunk so they overlap the next
         chunk's attention
"""
import json
import time

import numpy as np

import bass_rust
import concourse.bass as bass
import concourse.mybir as mybir
import concourse.tile as tile

F32 = mybir.dt.float32
F32R = mybir.dt.float32r
MULT = mybir.AluOpType.mult
ADD = mybir.AluOpType.add
SUB = mybir.AluOpType.subtract
AX = mybir.AxisListType.X
AF = mybir.ActivationFunctionType

B, S, D = 2, 2048, 1024
H, Hd = 16, 64
HPC = 4               # heads per core
THDC = HPC * Hd       # 256 channels per core
NT = S // 128         # 16 s-tiles
NCH = S // 512        # 4 chunks
DK = D // 128         # 8 contraction tiles
LAMBDA_INIT = 0.8
GN_EPS = 1e-5
ROPE_BASE = 10000.0
QK_SCALE = float(Hd) ** -0.5  # 0.125


def _split_multi_waits(nc):
    """This container's walrus rejects >1 sync wait per instruction. Hoist
    extra waits onto same-engine NoOps inserted right before the instruction
    (engine queues are in-order, so semantics are unchanged)."""
    d = json.loads(bass_rust.module_to_json_string(nc.m))
    ctr = 0
    for f in d["functions"]:
        for bb in f["blocks"]:
            out = []
            for inst in bb.get("instructions", []):
                si = inst.get("sync_info")
                waits = si.get("on_wait", []) if si else []
                if len(waits) > 1:
                    for w in waits[:-1]:
                        ctr += 1
                        out.append({
                            "debug": inst.get("debug", 0),
                            "engine": inst["engine"],
                            "ins": [],
                            "name": f"WSPLIT-{ctr}",
                            "opcode": "NoOp",
                            "outs": [],
                            "sync_info": {"on_update": [], "on_wait": [w]},
                        })
                    si["on_wait"] = [waits[-1]]
                out.append(inst)
            bb["instructions"] = out
    nc.m = bass_rust.module_from_json_string(json.dumps(d))


def _rope_tables():
    inv_freq = 1.0 / (ROPE_BASE ** (np.arange(0, Hd, 2, dtype=np.float32) / Hd))
    t = np.arange(S, dtype=np.float32)
    freqs = np.outer(t, inv_freq).astype(np.float32)
    emb = np.concatenate([freqs, freqs], axis=-1)       # (S, Hd)
    cos = np.cos(emb).astype(np.float32)
    sin = np.sin(emb).astype(np.float32)
    # sign-folded shifted-sin table: rotate_half contributes
    # out[d] = q[(d+32)%64] * g[d], g = [-sin[:32] | +sin[32:]]
    g = np.concatenate([-sin[:, :32], sin[:, 32:]], axis=1)
    return cos.T.copy(), g.T.copy()                     # (Hd, S) each


def _hview(t):
    """[128, 260] tile -> [128, 4, 64] per-head data columns."""
    return t[:, :].rearrange("p (h c) -> p h c", h=HPC)[:, :, 0:Hd]


def _rview(t):
    """[128, 260] tile -> [128, 4] per-head r (denominator) columns."""
    v = t[:, :].rearrange("p (h c) -> p h c", h=HPC)[:, :, Hd:Hd + 1]
    return v.rearrange("p h c -> p (h c)")


def _b64(ap):
    """[128, 4] -> [128, 4, 64] broadcast along a new inner dim."""
    return ap.unsqueeze(2).broadcast_to((128, HPC, Hd))


def build_module():
    nc = bass.Bass(trn_type="TRN2")

    x_d = nc.dram_tensor("x", [S, D], F32, kind="ExternalInput")
    wq_d = nc.dram_tensor("wq", [D, 512], F32, kind="ExternalInput")
    wk_d = nc.dram_tensor("wk", [D, 512], F32, kind="ExternalInput")
    wv_d = nc.dram_tensor("wv", [D, THDC], F32, kind="ExternalInput")
    wo_d = nc.dram_tensor("wo", [THDC, D], F32, kind="ExternalInput")
    lam_d = nc.dram_tensor("lam", [5, HPC], F32, kind="ExternalInput")
    gnw_d = nc.dram_tensor("gnw", [THDC], F32, kind="ExternalInput")
    gnb_d = nc.dram_tensor("gnb", [THDC], F32, kind="ExternalInput")
    cos_d = nc.dram_tensor("cosT", [Hd, S], F32, kind="ExternalInput")
    sin_d = nc.dram_tensor("ssinT", [Hd, S], F32, kind="ExternalInput")
    y_d = nc.dram_tensor("y", [S, D], F32, kind="ExternalOutput")
    ident_d = nc.inline_tensor(np.eye(128, dtype=np.float32), name="identity")

    dma = nc.sync.dma_start  # HWDGE: no Pool-engine descriptor-prep cost

    with tile.TileContext(nc) as tc:
        with tc.tile_pool(name="persist", bufs=1) as pA, \
             tc.tile_pool(name="dscr", bufs=1, space="DRAM") as pD:
            qt = [pA.tile([128, S], F32R, tag=f"qt{i}", name=f"qt{i}") for i in range(4)]
            kt = [pA.tile([128, S], F32R, tag=f"kt{i}", name=f"kt{i}") for i in range(4)]
            vaug = [pA.tile([128, HPC * (Hd + 1)], F32R, tag=f"va{i}", name=f"va{i}")
                    for i in range(NT)]
            ident = pA.tile([128, 128], F32, tag="ident", name="ident")
            dma(out=ident, in_=ident_d.ap())

            # ---- phase 0: lambdas, gn scale/bias prep ----
            lamsb = pA.tile([1, 20], F32, tag="lamsb", name="lamsb")
            dma(out=lamsb, in_=lam_d[:, :].rearrange("a b -> (a b)").unsqueeze(0))
            lt = pA.tile([1, 8], F32, tag="lt", name="lt")
            nc.vector.tensor_tensor(out=lt[:, 0:4], in0=lamsb[:, 0:4], in1=lamsb[:, 4:8], op=MULT)
            nc.vector.tensor_tensor(out=lt[:, 4:8], in0=lamsb[:, 8:12], in1=lamsb[:, 12:16], op=MULT)
            nc.scalar.activation(out=lt, in_=lt, func=AF.Exp)
            lamv = pA.tile([1, 4], F32, tag="lamv", name="lamv")
            nc.vector.tensor_tensor(out=lamv, in0=lt[:, 0:4], in1=lt[:, 4:8], op=SUB)
            nc.vector.tensor_tensor(out=lamv, in0=lamv, in1=lamsb[:, 16:20], op=ADD)
            slam = pA.tile([1, 1], F32, tag="slam", name="slam")
            nc.scalar.activation(out=slam, in_=lamsb[:, 16:17], func=AF.Identity,
                                 scale=-1.0, bias=1.0)
            dscr = pD.tile([1, 8], F32, name="dscr")
            dma(out=dscr[0:1, 0:4], in_=lamv)
            dma(out=dscr[0:1, 4:5], in_=slam)
            lam_b = pA.tile([128, 4], F32, tag="lam_b", name="lam_b")
            dma(out=lam_b, in_=dscr[0:1, 0:4].squeeze(0).partition_broadcast(128))
            slam_b = pA.tile([128, 1], F32, tag="slam_b", name="slam_b")
            dma(out=slam_b, in_=dscr[0:1, 4:5].squeeze(0).partition_broadcast(128))
            gnw_eff = pA.tile([128, THDC], F32, tag="gnw_eff", name="gnw_eff")
            dma(out=gnw_eff, in_=gnw_d.ap().partition_broadcast(128))
            nc.vector.tensor_scalar_mul(gnw_eff, gnw_eff, slam_b[:, 0:1])
            gnb_eff = pA.tile([128, THDC], F32, tag="gnb_eff", name="gnb_eff")
            dma(out=gnb_eff, in_=gnb_d.ap().partition_broadcast(128))
            nc.vector.tensor_scalar_mul(gnb_eff, gnb_eff, slam_b[:, 0:1])
            ones1 = pA.tile([128, 1], F32, tag="ones1", name="ones1")
            nc.vector.memset(ones1, 1.0)

            # ---- phases 1-2: x^T, V/K/Q projections, rope ----
            with tc.tile_pool(name="ph12", bufs=1) as pB, \
                 tc.tile_pool(name="xload", bufs=2) as pX, \
                 tc.tile_pool(name="wraw", bufs=3) as pWr, \
                 tc.tile_pool(name="wrow", bufs=8) as pW, \
                 tc.tile_pool(name="wvrow", bufs=8) as pWv, \
                 tc.tile_pool(name="trot", bufs=3) as pT, \
                 tc.tile_pool(name="ps_tr", bufs=4, space="PSUM") as pPtr, \
                 tc.tile_pool(name="ps_proj", bufs=2, space="PSUM") as pPproj:
                cos2h = pB.tile([128, S], F32, tag="cos2h", name="cos2h")
                sin2h = pB.tile([128, S], F32, tag="sin2h", name="sin2h")
                dma(out=cos2h[0:64, :], in_=cos_d[:, :])
                dma(out=cos2h[64:128, :], in_=cos_d[:, :])
                dma(out=sin2h[0:64, :], in_=sin_d[:, :])
                dma(out=sin2h[64:128, :], in_=sin_d[:, :])
                xT = [pB.tile([128, S // 2], F32R, tag=f"xT{i}", name=f"xT{i}")
                      for i in range(DK)]

                def rope(dst, tt_i, c, psq):
                    cs = slice(c * 512, (c + 1) * 512)
                    nc.vector.tensor_tensor(out=dst[tt_i][:, cs], in0=psq,
                                            in1=cos2h[:, cs], op=MULT)
                    trot = pT.tile([128, 512], F32, tag="trot", name=f"trot{tt_i}_{c}")
                    for hb in (0, 64):
                        nc.vector.tensor_tensor(
                            out=trot[hb:hb + 32, :], in0=psq[hb + 32:hb + 64, :],
                            in1=sin2h[hb:hb + 32, cs], op=MULT)
                        nc.vector.tensor_tensor(
                            out=trot[hb + 32:hb + 64, :], in0=psq[hb:hb + 32, :],
                            in1=sin2h[hb + 32:hb + 64, cs], op=MULT)
                    nc.vector.tensor_tensor(out=dst[tt_i][:, cs], in0=dst[tt_i][:, cs],
                                            in1=trot, op=ADD)

                for sg in range(2):
                    # x^T for this half (x loads split for earlier first transpose)
                    for stl in range(8):
                        st = sg * 8 + stl
                        xtile = pX.tile([128, D], F32, tag="xtile", name=f"xt{st}")
                        dma(out=xtile[:, 0:512], in_=x_d[st * 128:(st + 1) * 128, 0:512])
                        dma(out=xtile[:, 512:1024], in_=x_d[st * 128:(st + 1) * 128, 512:1024])
                        for dk in range(DK):
                            ptr = pPtr.tile([128, 128], F32, tag="ptr", name=f"ptx{st}_{dk}")
                            nc.tensor.transpose(ptr, xtile[:, dk * 128:(dk + 1) * 128], ident)
                            nc.scalar.copy(out=xT[dk][:, stl * 128:(stl + 1) * 128], in_=ptr)

                    # V projection first (attention is gated on it)
                    wvrow = []
                    for dk in range(DK):
                        raw = pWr.tile([128, THDC], F32, tag="wvraw", name=f"wvr{sg}{dk}")
                        dma(out=raw, in_=wv_d[dk * 128:(dk + 1) * 128, :])
                        wr = pWv.tile([128, THDC], F32R, tag="wvrow", name=f"wv{sg}{dk}")
                        nc.vector.tensor_copy(out=wr, in_=raw)
                        wvrow.append(wr)
                    for stl in range(8):
                        st = sg * 8 + stl
                        psv = pPproj.tile([128, THDC], F32, tag="psq", name=f"psv{st}")
                        for i in range(DK):
                            dk = (i + stl) % DK
                            nc.tensor.matmul(psv, lhsT=xT[dk][:, stl * 128:(stl + 1) * 128],
                                             rhs=wvrow[dk], start=(i == 0), stop=(i == DK - 1))
                        va = vaug[st]
                        vav = va[:, :].rearrange("p (h c) -> p h c", h=HPC)
                        nc.vector.tensor_copy(out=vav[:, :, 0:Hd],
                                              in_=psv[:, :].rearrange("p (h c) -> p h c", h=HPC))
                        for h in range(HPC):
                            nc.vector.tensor_copy(out=va[:, h * 65 + 64:h * 65 + 65], in_=ones1)

                    # K then Q projections + rope; heads 0/1 tiles (tt 0,2) first
                    for which, (w_dram, dst) in enumerate(((wk_d, kt), (wq_d, qt))):
                        wrow = []
                        for dk in range(DK):
                            raw = pWr.tile([128, 512], F32, tag="wraw", name=f"wr{sg}{which}{dk}")
                            dma(out=raw, in_=w_dram[dk * 128:(dk + 1) * 128, :])
                            wr = pW.tile([128, 512], F32R, tag="wrow", name=f"w{sg}{which}{dk}")
                            nc.vector.tensor_copy(out=wr, in_=raw)
                            wrow.append(wr)
                        for tt_i in (0, 2, 1, 3):
                            for cl in range(2):
                                c = sg * 2 + cl
                                psq = pPproj.tile([128, 512], F32, tag="psq",
                                                  name=f"psq{which}{tt_i}_{c}")
                                for i in range(DK):
                                    dk = (i + tt_i * 2 + cl) % DK
                                    nc.tensor.matmul(
                                        psq, lhsT=wrow[dk][:, tt_i * 128:(tt_i + 1) * 128],
                                        rhs=xT[dk][:, cl * 512:(cl + 1) * 512],
                                        start=(i == 0), stop=(i == DK - 1))
                                rope(dst, tt_i, c, psq)

            # ---- phases 3-5: attention + group norm + o_proj, chunk-major ----
            with tc.tile_pool(name="owyt", bufs=1) as pC0, \
                 tc.tile_pool(name="etile", bufs=3) as pE, \
                 tc.tile_pool(name="osb", bufs=4) as pO, \
                 tc.tile_pool(name="gn", bufs=2) as pG, \
                 tc.tile_pool(name="outsb", bufs=3) as pOut, \
                 tc.tile_pool(name="wosb", bufs=1) as pWo, \
                 tc.tile_pool(name="ps_s", bufs=2, space="PSUM") as pPs, \
                 tc.tile_pool(name="ps_o", bufs=2, space="PSUM") as pPo, \
                 tc.tile_pool(name="ps_t2", bufs=2, space="PSUM") as pPt2:
                ow = [[pC0.tile([128, HPC * (Hd + 1)], F32, tag=f"ow{br}_{i}",
                                name=f"ow{br}_{i}") for i in range(NT)] for br in range(2)]
                yt = [pC0.tile([128, S], F32R, tag=f"yt{i}", name=f"yt{i}")
                      for i in range(2)]
                wo_sb = []
                for ci in range(2):
                    raw = pWo.tile([128, D], F32, tag=f"woraw{ci}", name=f"woraw{ci}")
                    dma(out=raw, in_=wo_d[ci * 128:(ci + 1) * 128, :])
                    wr = pWo.tile([128, D], F32R, tag=f"wo{ci}", name=f"wo{ci}")
                    nc.vector.tensor_copy(out=wr, in_=raw)
                    wo_sb.append(wr)

                for c in range(NCH):
                    cs = slice(c * 512, (c + 1) * 512)
                    for h in range(HPC):
                        tt1 = h // 2
                        tt2 = 2 + h // 2
                        ro = (h % 2) * 64
                        pso = [pPo.tile([65, 512], F32, tag="pso", name=f"pso{c}{h}{br}")
                               for br in range(2)]
                        for sk in range(NT):
                            pss = pPs.tile([128, 1024], F32, tag="pss",
                                           name=f"pss{c}{h}{sk}")
                            nc.tensor.matmul(pss[:, 0:512],
                                             lhsT=kt[tt1][ro:ro + 64, sk * 128:(sk + 1) * 128],
                                             rhs=qt[tt1][ro:ro + 64, cs],
                                             start=True, stop=True)
                            nc.tensor.matmul(pss[:, 512:1024],
                                             lhsT=kt[tt2][ro:ro + 64, sk * 128:(sk + 1) * 128],
                                             rhs=qt[tt2][ro:ro + 64, cs],
                                             start=True, stop=True)
                            e = pE.tile([128, 1024], F32R, tag="e", name=f"e{c}{h}{sk}")
                            nc.scalar.activation(out=e, in_=pss, func=AF.Exp,
                                                 scale=QK_SCALE)
                            va = vaug[sk][:, h * 65:(h + 1) * 65]
                            nc.tensor.matmul(pso[0], lhsT=va, rhs=e[:, 0:512],
                                             start=(sk == 0), stop=(sk == NT - 1))
                            nc.tensor.matmul(pso[1], lhsT=va, rhs=e[:, 512:1024],
                                             start=(sk == 0), stop=(sk == NT - 1))
                        for br in range(2):
                            osb = pO.tile([65, 512], F32, tag="osb", name=f"osb{c}{h}{br}")
                            nc.vector.tensor_copy(out=osb, in_=pso[br])
                            for stl in range(4):
                                st = c * 4 + stl
                                ptr = pPt2.tile([128, 65], F32, tag="ptr2",
                                                name=f"po{c}{h}{br}{stl}")
                                nc.tensor.transpose(ptr, osb[:, stl * 128:(stl + 1) * 128],
                                                    ident[0:65, 0:65])
                                nc.vector.tensor_copy(out=ow[br][st][:, h * 65:(h + 1) * 65],
                                                      in_=ptr)

                    # group norm + y^T + o_proj for this chunk's 4 s-tiles
                    for stl in range(4):
                        st = c * 4 + stl
                        ow1, ow2 = ow[0][st], ow[1][st]
                        rec = pG.tile([128, 4], F32, tag="rec", name=f"rec{st}")
                        nc.vector.reciprocal(rec, _rview(ow2))
                        rho = pG.tile([128, 4], F32, tag="rho", name=f"rho{st}")
                        nc.vector.tensor_tensor(out=rho, in0=_rview(ow1), in1=rec, op=MULT)
                        nc.vector.tensor_tensor(out=rho, in0=rho, in1=lam_b, op=MULT)
                        dt_ = pG.tile([128, THDC], F32, tag="dt", name=f"dt{st}")
                        dtv = dt_[:, :].rearrange("p (h c) -> p h c", h=HPC)
                        nc.vector.tensor_tensor(out=dtv, in0=_hview(ow2), in1=_b64(rho), op=MULT)
                        nc.vector.tensor_tensor(out=dtv, in0=_hview(ow1), in1=dtv, op=SUB)
                        s1 = pG.tile([128, 4], F32, tag="s1", name=f"s1{st}")
                        nc.vector.reduce_sum(out=s1, in_=dtv, axis=AX)
                        nc.vector.tensor_scalar_mul(s1, s1, -1.0 / Hd)
                        nc.vector.tensor_tensor(out=dtv, in0=dtv, in1=_b64(s1), op=ADD)
                        d2 = pG.tile([128, THDC], F32, tag="d2", name=f"d2{st}")
                        nc.gpsimd.tensor_tensor(out=d2, in0=dt_, in1=dt_, op=MULT)
                        s2 = pG.tile([128, 4], F32, tag="s2", name=f"s2{st}")
                        nc.vector.reduce_sum(out=s2,
                                             in_=d2[:, :].rearrange("p (h c) -> p h c", h=HPC),
                                             axis=AX)
                        nc.vector.tensor_tensor(out=rec, in0=_rview(ow1), in1=_rview(ow1), op=MULT)
                        nc.vector.tensor_scalar_mul(rec, rec, GN_EPS)
                        nc.vector.tensor_scalar_mul(s2, s2, 1.0 / Hd)
                        nc.vector.tensor_tensor(out=s2, in0=s2, in1=rec, op=ADD)
                        nc.scalar.activation(out=s2, in_=s2, func=AF.Sqrt)
                        nc.vector.reciprocal(s2, s2)
                        nc.vector.tensor_tensor(out=dtv, in0=dtv, in1=_b64(s2), op=MULT)
                        nc.gpsimd.tensor_tensor(out=dt_, in0=dt_, in1=gnw_eff, op=MULT)
                        nc.gpsimd.tensor_tensor(out=dt_, in0=dt_, in1=gnb_eff, op=ADD)
                        for ci in range(2):
                            ptr = pPt2.tile([128, 128], F32, tag="ptr2", name=f"py{st}_{ci}")
                            nc.tensor.transpose(ptr, dt_[:, ci * 128:(ci + 1) * 128], ident)
                            nc.vector.tensor_copy(out=yt[ci][:, st * 128:(st + 1) * 128],
                                                  in_=ptr)
                        for oc in range(2):
                            pout = pPo.tile([128, 512], F32, tag="pso", name=f"pout{st}_{oc}")
                            for ci in range(2):
                                nc.tensor.matmul(pout,
                                                 lhsT=yt[ci][:, st * 128:(st + 1) * 128],
                                                 rhs=wo_sb[ci][:, oc * 512:(oc + 1) * 512],
                                                 start=(ci == 0), stop=(ci == 1))
                            ost = pOut.tile([128, 512], F32, tag="ost", name=f"ost{st}_{oc}")
                            nc.vector.tensor_copy(out=ost, in_=pout)
                            dma(out=y_d[st * 128:(st + 1) * 128, oc * 512:(oc + 1) * 512],
                                in_=ost)

    _split_multi_waits(nc)
    return nc


_CACHE = {}


def _get_module():
    if "nc" not in _CACHE:
        _CACHE["nc"] = build_module()
        _CACHE["tables"] = _rope_tables()
    return _CACHE["nc"], _CACHE["tables"]


def kernel(x, Wq, Wk, Wv, Wo, lambda_q1, lambda_k1, lambda_q2, lambda_k2,
           lambda_init, gn_weight, gn_bias):
    from concourse.bass_utils import run_bass_kernel_spmd

    x = np.ascontiguousarray(np.asarray(x, dtype=np.float32))
    Wq = np.asarray(Wq, dtype=np.float32)
    Wk = np.asarray(Wk, dtype=np.float32)
    Wv = np.asarray(Wv, dtype=np.float32)
    Wo = np.asarray(Wo, dtype=np.float32)
    lq1 = np.asarray(lambda_q1, dtype=np.float32)
    lk1 = np.asarray(lambda_k1, dtype=np.float32)
    lq2 = np.asarray(lambda_q2, dtype=np.float32)
    lk2 = np.asarray(lambda_k2, dtype=np.float32)
    lam_init = np.float32(np.asarray(lambda_init).reshape(()))
    gnw = np.asarray(gn_weight, dtype=np.float32)
    gnb = np.asarray(gn_bias, dtype=np.float32)

    nc, (cosT, ssinT) = _get_module()

    in_maps = []
    for core in range(8):
        b = core // 4
        hb = (core % 4) * HPC
        c1 = slice(hb * Hd, (hb + HPC) * Hd)
        c2 = slice(H * Hd + hb * Hd, H * Hd + (hb + HPC) * Hd)
        lam = np.stack([lq1[hb:hb + HPC], lk1[hb:hb + HPC],
                        lq2[hb:hb + HPC], lk2[hb:hb + HPC],
                        np.full(HPC, lam_init, np.float32)]).astype(np.float32)
        in_maps.append({
            "x": np.ascontiguousarray(x[b]),
            "wq": np.ascontiguousarray(np.concatenate([Wq[:, c1], Wq[:, c2]], axis=1)),
            "wk": np.ascontiguousarray(np.concatenate([Wk[:, c1], Wk[:, c2]], axis=1)),
            "wv": np.ascontiguousarray(Wv[:, c1]),
            "wo": np.ascontiguousarray(Wo[c1, :]),
            "lam": lam,
            "gnw": np.ascontiguousarray(gnw[c1]),
            "gnb": np.ascontiguousarray(gnb[c1]),
            "cosT": cosT,
            "ssinT": ssinT,
        })

    last_err = None
    for attempt in range(3):
        try:
            res = run_bass_kernel_spmd(nc, in_maps, core_ids=list(range(8)))
            break
        except Exception as e:  # transient axon/device hiccups
            last_err = e
            time.sleep(10 * (attempt + 1))
    else:
        raise last_err

    out = np.zeros((B, S, D), dtype=np.float32)
    for core in range(8):
        out[core // 4] += res.results[core]["y"]
    return out


# revision 6
# speedup vs baseline: 1.1150x; 1.1150x over previous
"""Differential attention (nn_DifferentialAttention_84679575208071) on 8 TRN2
NeuronCores via Bass/Tile.

Sharding: hybrid data-parallel x tensor-parallel. Core c handles batch c//4 and
heads 4*(c%4) .. 4*(c%4)+4 (B=2, H=16 -> 2 batch groups x 4-way head split).
Each core computes its 4 heads' attention + group-norm + partial o_proj; the
host sums the 4 partial (S, D) outputs per batch (tensor-parallel o_proj
reduction) and stacks the 2 batches.

Per-core algorithm (all matmuls f32r = full-rate PE):
  xT = x^T (PE transposes)                        [D, S]
  qT/kT = (Wq/Wk slice)^T x^T with RoPE applied   [512, S] (2 branches x 4 heads x 64)
  v    = x Wv slice (row layout)                  [S, 256], augmented with a
         ones column per head -> PV matmul also yields softmax denominators
  attention, chunk-major: per (sq-chunk, head): both branches' score tiles go
         into one [128,1024] PSUM -> one wide exp (no max subtraction; scores
         are O(+-6) so fp32 exp is exact-safe) -> two PV matmuls [V|1]^T E^T
  diff = O1 - (lam*r1/r2) * O2  (group-norm is scale-invariant up to eps, so
         normalizing by r1 is folded in; eps is scaled by r1^2 to compensate)
  group-norm over each head's 64 channels, * gn_weight*(1-lam_init) + bias,
         then o_proj — both streamed per chunk so they overlap the next
         chunk's attention
"""
import json
import time

import numpy as np

import bass_rust
import concourse.bass as bass
import concourse.mybir as mybir
import concourse.tile as tile

F32 = mybir.dt.float32
F32R = mybir.dt.float32r
MULT = mybir.AluOpType.mult
ADD = mybir.AluOpType.add
SUB = mybir.AluOpType.subtract
AX = mybir.AxisListType.X
AF = mybir.ActivationFunctionType

B, S, D = 2, 2048, 1024
H, Hd = 16, 64
HPC = 4               # heads per core
THDC = HPC * Hd       # 256 channels per core
NT = S // 128         # 16 s-tiles
NCH = S // 512        # 4 chunks
DK = D // 128         # 8 contraction tiles
LAMBDA_INIT = 0.8
GN_EPS = 1e-5
ROPE_BASE = 10000.0
QK_SCALE = float(Hd) ** -0.5  # 0.125


def _split_multi_waits(nc):
    """This container's walrus rejects >1 sync wait per instruction. Hoist
    extra waits onto same-engine NoOps inserted right before the instruction
    (engine queues are in-order, so semantics are unchanged)."""
    d = json.loads(bass_rust.module_to_json_string(nc.m))
    ctr = 0
    for f in d["functions"]:
        for bb in f["blocks"]:
            out = []
            for inst in bb.get("instructions", []):
                si = inst.get("sync_info")
                waits = si.get("on_wait", []) if si else []
                if len(waits) > 1:
                    for w in waits[:-1]:
                        ctr += 1
                        out.append({
                            "debug": inst.get("debug", 0),
                            "engine": inst["engine"],
                            "ins": [],
                            "name": f"WSPLIT-{ctr}",
                            "opcode": "NoOp",
                            "outs": [],
                            "sync_info": {"on_update": [], "on_wait": [w]},
                        })
                    si["on_wait"] = [waits[-1]]
                out.append(inst)
            bb["instructions"] = out
    nc.m = bass_rust.module_from_json_string(json.dumps(d))


def _rope_tables():
    inv_freq = 1.0 / (ROPE_BASE ** (np.arange(0, Hd, 2, dtype=np.float32) / Hd))
    t = np.arange(S, dtype=np.float32)
    freqs = np.outer(t, inv_freq).astype(np.float32)
    emb = np.concatenate([freqs, freqs], axis=-1)       # (S, Hd)
    cos = np.cos(emb).astype(np.float32)
    sin = np.sin(emb).astype(np.float32)
    # sign-folded shifted-sin table: rotate_half contributes
    # out[d] = q[(d+32)%64] * g[d], g = [-sin[:32] | +sin[32:]]
    g = np.concatenate([-sin[:, :32], sin[:, 32:]], axis=1)
    return cos.T.copy(), g.T.copy()                     # (Hd, S) each


def _hview(t):
    """[128, 260] tile -> [128, 4, 64] per-head data columns."""
    return t[:, :].rearrange("p (h c) -> p h c", h=HPC)[:, :, 0:Hd]


def _rview(t):
    """[128, 260] tile -> [128, 4] per-head r (denominator) columns."""
    v = t[:, :].rearrange("p (h c) -> p h c", h=HPC)[:, :, Hd:Hd + 1]
    return v.rearrange("p h c -> p (h c)")


def _b64(ap):
    """[128, 4] -> [128, 4, 64] broadcast along a new inner dim."""
    return ap.unsqueeze(2).broadcast_to((128, HPC, Hd))


def build_module():
    nc = bass.Bass(trn_type="TRN2")

    x_d = nc.dram_tensor("x", [S, D], F32, kind="ExternalInput")
    wq_d = nc.dram_tensor("wq", [D, 512], F32, kind="ExternalInput")
    wk_d = nc.dram_tensor("wk", [D, 512], F32, kind="ExternalInput")
    wv_d = nc.dram_tensor("wv", [D, THDC], F32, kind="ExternalInput")
    wo_d = nc.dram_tensor("wo", [THDC, D], F32, kind="ExternalInput")
    lam_d = nc.dram_tensor("lam", [5, HPC], F32, kind="ExternalInput")
    gnw_d = nc.dram_tensor("gnw", [THDC], F32, kind="ExternalInput")
    gnb_d = nc.dram_tensor("gnb", [THDC], F32, kind="ExternalInput")
    cos_d = nc.dram_tensor("cosT", [Hd, S], F32, kind="ExternalInput")
    sin_d = nc.dram_tensor("ssinT", [Hd, S], F32, kind="ExternalInput")
    y_d = nc.dram_tensor("y", [S, D], F32, kind="ExternalOutput")
    ident_d = nc.inline_tensor(np.eye(128, dtype=np.float32), name="identity")

    dma = nc.sync.dma_start  # HWDGE: no Pool-engine descriptor-prep cost

    with tile.TileContext(nc) as tc:
        with tc.tile_pool(name="persist", bufs=1) as pA, \
             tc.tile_pool(name="dscr", bufs=1, space="DRAM") as pD:
            qt = [pA.tile([128, S], F32R, tag=f"qt{i}", name=f"qt{i}") for i in range(4)]
            kt = [pA.tile([128, S], F32R, tag=f"kt{i}", name=f"kt{i}") for i in range(4)]
            vaug = [pA.tile([128, HPC * (Hd + 1)], F32R, tag=f"va{i}", name=f"va{i}")
                    for i in range(NT)]
            ident = pA.tile([128, 128], F32, tag="ident", name="ident")
            dma(out=ident, in_=ident_d.ap())

            # ---- phase 0: lambdas, gn scale/bias prep ----
            lamsb = pA.tile([1, 20], F32, tag="lamsb", name="lamsb")
            dma(out=lamsb, in_=lam_d[:, :].rearrange("a b -> (a b)").unsqueeze(0))
            lt = pA.tile([1, 8], F32, tag="lt", name="lt")
            nc.vector.tensor_tensor(out=lt[:, 0:4], in0=lamsb[:, 0:4], in1=lamsb[:, 4:8], op=MULT)
            nc.vector.tensor_tensor(out=lt[:, 4:8], in0=lamsb[:, 8:12], in1=lamsb[:, 12:16], op=MULT)
            nc.scalar.activation(out=lt, in_=lt, func=AF.Exp)
            lamv = pA.tile([1, 4], F32, tag="lamv", name="lamv")
            nc.vector.tensor_tensor(out=lamv, in0=lt[:, 0:4], in1=lt[:, 4:8], op=SUB)
            nc.vector.tensor_tensor(out=lamv, in0=lamv, in1=lamsb[:, 16:20], op=ADD)
            slam = pA.tile([1, 1], F32, tag="slam", name="slam")
            nc.scalar.activation(out=slam, in_=lamsb[:, 16:17], func=AF.Identity,
                                 scale=-1.0, bias=1.0)
            dscr = pD.tile([1, 8], F32, name="dscr")
            dma(out=dscr[0:1, 0:4], in_=lamv)
            dma(out=dscr[0:1, 4:5], in_=slam)
            lam_b = pA.tile([128, 4], F32, tag="lam_b", name="lam_b")
            dma(out=lam_b, in_=dscr[0:1, 0:4].squeeze(0).partition_broadcast(128))
            slam_b = pA.tile([128, 1], F32, tag="slam_b", name="slam_b")
            dma(out=slam_b, in_=dscr[0:1, 4:5].squeeze(0).partition_broadcast(128))
            gnw_eff = pA.tile([128, THDC], F32, tag="gnw_eff", name="gnw_eff")
            dma(out=gnw_eff, in_=gnw_d.ap().partition_broadcast(128))
            nc.vector.tensor_scalar_mul(gnw_eff, gnw_eff, slam_b[:, 0:1])
            gnb_eff = pA.tile([128, THDC], F32, tag="gnb_eff", name="gnb_eff")
            dma(out=gnb_eff, in_=gnb_d.ap().partition_broadcast(128))
            nc.vector.tensor_scalar_mul(gnb_eff, gnb_eff, slam_b[:, 0:1])
            ones1 = pA.tile([128, 1], F32, tag="ones1", name="ones1")
            nc.vector.memset(ones1, 1.0)

            # ---- phases 1-2: x^T, V/K/Q projections, rope ----
            with tc.tile_pool(name="ph12", bufs=1) as pB, \
                 tc.tile_pool(name="xload", bufs=2) as pX, \
                 tc.tile_pool(name="wraw", bufs=3) as pWr, \
                 tc.tile_pool(name="wrow", bufs=8) as pW, \
                 tc.tile_pool(name="wvrow", bufs=8) as pWv, \
                 tc.tile_pool(name="trot", bufs=3) as pT, \
                 tc.tile_pool(name="ps_tr", bufs=4, space="PSUM") as pPtr, \
                 tc.tile_pool(name="ps_proj", bufs=2, space="PSUM") as pPproj:
                cos2h = pB.tile([128, S], F32, tag="cos2h", name="cos2h")
                sin2h = pB.tile([128, S], F32, tag="sin2h", name="sin2h")
                dma(out=cos2h[0:64, :], in_=cos_d[:, :])
                dma(out=cos2h[64:128, :], in_=cos_d[:, :])
                dma(out=sin2h[0:64, :], in_=sin_d[:, :])
                dma(out=sin2h[64:128, :], in_=sin_d[:, :])
                xT = [pB.tile([128, S // 2], F32R, tag=f"xT{i}", name=f"xT{i}")
                      for i in range(DK)]

                def rope(dst, tt_i, c, psq):
                    cs = slice(c * 512, (c + 1) * 512)
                    nc.vector.tensor_tensor(out=dst[tt_i][:, cs], in0=psq,
                                            in1=cos2h[:, cs], op=MULT)
                    trot = pT.tile([128, 512], F32, tag="trot", name=f"trot{tt_i}_{c}")
                    for hb in (0, 64):
                        nc.vector.tensor_tensor(
                            out=trot[hb:hb + 32, :], in0=psq[hb + 32:hb + 64, :],
                            in1=sin2h[hb:hb + 32, cs], op=MULT)
                        nc.vector.tensor_tensor(
                            out=trot[hb + 32:hb + 64, :], in0=psq[hb:hb + 32, :],
                            in1=sin2h[hb + 32:hb + 64, cs], op=MULT)
                    nc.vector.tensor_tensor(out=dst[tt_i][:, cs], in0=dst[tt_i][:, cs],
                                            in1=trot, op=ADD)

                for sg in range(2):
                    # x^T for this half (x loads split for earlier first transpose)
                    for stl in range(8):
                        st = sg * 8 + stl
                        xtile = pX.tile([128, D], F32, tag="xtile", name=f"xt{st}")
                        dma(out=xtile[:, 0:512], in_=x_d[st * 128:(st + 1) * 128, 0:512])
                        dma(out=xtile[:, 512:1024], in_=x_d[st * 128:(st + 1) * 128, 512:1024])
                        for dk in range(DK):
                            ptr = pPtr.tile([128, 128], F32, tag="ptr", name=f"ptx{st}_{dk}")
                            nc.tensor.transpose(ptr, xtile[:, dk * 128:(dk + 1) * 128], ident)
                            nc.scalar.copy(out=xT[dk][:, stl * 128:(stl + 1) * 128], in_=ptr)

                    # V projection first (attention is gated on it)
                    wvrow = []
                    for dk in range(DK):
                        raw = pWr.tile([128, THDC], F32, tag="wvraw", name=f"wvr{sg}{dk}")
                        dma(out=raw, in_=wv_d[dk * 128:(dk + 1) * 128, :])
                        wr = pWv.tile([128, THDC], F32R, tag="wvrow", name=f"wv{sg}{dk}")
                        nc.vector.tensor_copy(out=wr, in_=raw)
                        wvrow.append(wr)
                    for stl in range(8):
                        st = sg * 8 + stl
                        psv = pPproj.tile([128, THDC], F32, tag="psq", name=f"psv{st}")
                        for i in range(DK):
                            dk = (i + stl) % DK
                            nc.tensor.matmul(psv, lhsT=xT[dk][:, stl * 128:(stl + 1) * 128],
                                             rhs=wvrow[dk], start=(i == 0), stop=(i == DK - 1))
                        va = vaug[st]
                        vav = va[:, :].rearrange("p (h c) -> p h c", h=HPC)
                        nc.vector.tensor_copy(out=vav[:, :, 0:Hd],
                                              in_=psv[:, :].rearrange("p (h c) -> p h c", h=HPC))
                        for h in range(HPC):
                            nc.vector.tensor_copy(out=va[:, h * 65 + 64:h * 65 + 65], in_=ones1)

                    # K then Q projections + rope; heads 0/1 tiles (tt 0,2) first
                    for which, (w_dram, dst) in enumerate(((wk_d, kt), (wq_d, qt))):
                        wrow = []
                        for dk in range(DK):
                            raw = pWr.tile([128, 512], F32, tag="wraw", name=f"wr{sg}{which}{dk}")
                            dma(out=raw, in_=w_dram[dk * 128:(dk + 1) * 128, :])
                            wr = pW.tile([128, 512], F32R, tag="wrow", name=f"w{sg}{which}{dk}")
                            nc.vector.tensor_copy(out=wr, in_=raw)
                            wrow.append(wr)
                        for tt_i in (0, 2, 1, 3):
                            for cl in range(2):
                                c = sg * 2 + cl
                                psq = pPproj.tile([128, 512], F32, tag="psq",
                                                  name=f"psq{which}{tt_i}_{c}")
                                for i in range(DK):
                                    dk = (i + tt_i * 2 + cl) % DK
                                    nc.tensor.matmul(
                                        psq, lhsT=wrow[dk][:, tt_i * 128:(tt_i + 1) * 128],
                                        rhs=xT[dk][:, cl * 512:(cl + 1) * 512],
                                        start=(i == 0), stop=(i == DK - 1))
                                rope(dst, tt_i, c, psq)

            # ---- phases 3-5: attention + group norm + o_proj, chunk-major ----
            with tc.tile_pool(name="owyt", bufs=1) as pC0, \
                 tc.tile_pool(name="etile", bufs=4) as pE, \
                 tc.tile_pool(name="osb", bufs=4) as pO, \
                 tc.tile_pool(name="gn", bufs=2) as pG, \
                 tc.tile_pool(name="outsb", bufs=3) as pOut, \
                 tc.tile_pool(name="wosb", bufs=1) as pWo, \
                 tc.tile_pool(name="ps_s", bufs=3, space="PSUM") as pPs, \
                 tc.tile_pool(name="ps_o", bufs=2, space="PSUM") as pPo, \
                 tc.tile_pool(name="ps_out", bufs=1, space="PSUM") as pPout, \
                 tc.tile_pool(name="ps_t2", bufs=2, space="PSUM") as pPt2:
                ow = [[pC0.tile([128, HPC * (Hd + 1)], F32, tag=f"ow{br}_{i}",
                                name=f"ow{br}_{i}") for i in range(NT)] for br in range(2)]
                yt = [pC0.tile([128, S], F32R, tag=f"yt{i}", name=f"yt{i}")
                      for i in range(2)]
                wo_sb = []
                for ci in range(2):
                    raw = pWo.tile([128, D], F32, tag=f"woraw{ci}", name=f"woraw{ci}")
                    dma(out=raw, in_=wo_d[ci * 128:(ci + 1) * 128, :])
                    wr = pWo.tile([128, D], F32R, tag=f"wo{ci}", name=f"wo{ci}")
                    nc.vector.tensor_copy(out=wr, in_=raw)
                    wo_sb.append(wr)

                for c in range(NCH):
                    cs = slice(c * 512, (c + 1) * 512)
                    for h in range(HPC):
                        tt1 = h // 2
                        tt2 = 2 + h // 2
                        ro = (h % 2) * 64
                        pso = [pPo.tile([65, 512], F32, tag="pso", name=f"pso{c}{h}{br}")
                               for br in range(2)]
                        for sk in range(NT):
                            va = vaug[sk][:, h * 65:(h + 1) * 65]
                            for br, tt_i in ((0, tt1), (1, tt2)):
                                pss = pPs.tile([128, 512], F32, tag="pss",
                                               name=f"pss{c}{h}{sk}{br}")
                                nc.tensor.matmul(pss,
                                                 lhsT=kt[tt_i][ro:ro + 64, sk * 128:(sk + 1) * 128],
                                                 rhs=qt[tt_i][ro:ro + 64, cs],
                                                 start=True, stop=True)
                                e = pE.tile([128, 512], F32R, tag="e", name=f"e{c}{h}{sk}{br}")
                                nc.scalar.activation(out=e, in_=pss, func=AF.Exp,
                                                     scale=QK_SCALE)
                                nc.tensor.matmul(pso[br], lhsT=va, rhs=e,
                                                 start=(sk == 0), stop=(sk == NT - 1))
                        for br in range(2):
                            osb = pO.tile([65, 512], F32, tag="osb", name=f"osb{c}{h}{br}")
                            nc.vector.tensor_copy(out=osb, in_=pso[br])
                            for stl in range(4):
                                st = c * 4 + stl
                                ptr = pPt2.tile([128, 65], F32, tag="ptr2",
                                                name=f"po{c}{h}{br}{stl}")
                                nc.tensor.transpose(ptr, osb[:, stl * 128:(stl + 1) * 128],
                                                    ident[0:65, 0:65])
                                nc.vector.tensor_copy(out=ow[br][st][:, h * 65:(h + 1) * 65],
                                                      in_=ptr)

                    # group norm + y^T + o_proj for this chunk's 4 s-tiles
                    for stl in range(4):
                        st = c * 4 + stl
                        ow1, ow2 = ow[0][st], ow[1][st]
                        rec = pG.tile([128, 4], F32, tag="rec", name=f"rec{st}")
                        nc.vector.reciprocal(rec, _rview(ow2))
                        rho = pG.tile([128, 4], F32, tag="rho", name=f"rho{st}")
                        nc.vector.tensor_tensor(out=rho, in0=_rview(ow1), in1=rec, op=MULT)
                        nc.vector.tensor_tensor(out=rho, in0=rho, in1=lam_b, op=MULT)
                        dt_ = pG.tile([128, THDC], F32, tag="dt", name=f"dt{st}")
                        dtv = dt_[:, :].rearrange("p (h c) -> p h c", h=HPC)
                        nc.vector.tensor_tensor(out=dtv, in0=_hview(ow2), in1=_b64(rho), op=MULT)
                        nc.vector.tensor_tensor(out=dtv, in0=_hview(ow1), in1=dtv, op=SUB)
                        s1 = pG.tile([128, 4], F32, tag="s1", name=f"s1{st}")
                        nc.vector.reduce_sum(out=s1, in_=dtv, axis=AX)
                        nc.vector.tensor_scalar_mul(s1, s1, -1.0 / Hd)
                        nc.vector.tensor_tensor(out=dtv, in0=dtv, in1=_b64(s1), op=ADD)
                        d2 = pG.tile([128, THDC], F32, tag="d2", name=f"d2{st}")
                        nc.gpsimd.tensor_tensor(out=d2, in0=dt_, in1=dt_, op=MULT)
                        s2 = pG.tile([128, 4], F32, tag="s2", name=f"s2{st}")
                        nc.vector.reduce_sum(out=s2,
                                             in_=d2[:, :].rearrange("p (h c) -> p h c", h=HPC),
                                             axis=AX)
                        nc.vector.tensor_tensor(out=rec, in0=_rview(ow1), in1=_rview(ow1), op=MULT)
                        nc.vector.tensor_scalar_mul(rec, rec, GN_EPS)
                        nc.vector.tensor_scalar_mul(s2, s2, 1.0 / Hd)
                        nc.vector.tensor_tensor(out=s2, in0=s2, in1=rec, op=ADD)
                        nc.scalar.activation(out=s2, in_=s2, func=AF.Sqrt)
                        nc.vector.reciprocal(s2, s2)
                        nc.vector.tensor_tensor(out=dtv, in0=dtv, in1=_b64(s2), op=MULT)
                        nc.gpsimd.tensor_tensor(out=dt_, in0=dt_, in1=gnw_eff, op=MULT)
                        nc.gpsimd.tensor_tensor(out=dt_, in0=dt_, in1=gnb_eff, op=ADD)
                        for ci in range(2):
                            ptr = pPt2.tile([128, 128], F32, tag="ptr2", name=f"py{st}_{ci}")
                            nc.tensor.transpose(ptr, dt_[:, ci * 128:(ci + 1) * 128], ident)
                            nc.vector.tensor_copy(out=yt[ci][:, st * 128:(st + 1) * 128],
                                                  in_=ptr)
                        for oc in range(2):
                            pout = pPout.tile([128, 512], F32, tag="pout", name=f"pout{st}_{oc}")
                            for ci in range(2):
                                nc.tensor.matmul(pout,
                                                 lhsT=yt[ci][:, st * 128:(st + 1) * 128],
                                                 rhs=wo_sb[ci][:, oc * 512:(oc + 1) * 512],
                                                 start=(ci == 0), stop=(ci == 1))
                            ost = pOut.tile([128, 512], F32, tag="ost", name=f"ost{st}_{oc}")
                            nc.vector.tensor_copy(out=ost, in_=pout)
                            dma(out=y_d[st * 128:(st + 1) * 128, oc * 512:(oc + 1) * 512],
                                in_=ost)

    _split_multi_waits(nc)
    return nc


_CACHE = {}


def _get_module():
    if "nc" not in _CACHE:
        _CACHE["nc"] = build_module()
        _CACHE["tables"] = _rope_tables()
    return _CACHE["nc"], _CACHE["tables"]


def kernel(x, Wq, Wk, Wv, Wo, lambda_q1, lambda_k1, lambda_q2, lambda_k2,
           lambda_init, gn_weight, gn_bias):
    from concourse.bass_utils import run_bass_kernel_spmd

    x = np.ascontiguousarray(np.asarray(x, dtype=np.float32))
    Wq = np.asarray(Wq, dtype=np.float32)
    Wk = np.asarray(Wk, dtype=np.float32)
    Wv = np.asarray(Wv, dtype=np.float32)
    Wo = np.asarray(Wo, dtype=np.float32)
    lq1 = np.asarray(lambda_q1, dtype=np.float32)
    lk1 = np.asarray(lambda_k1, dtype=np.float32)
    lq2 = np.asarray(lambda_q2, dtype=np.float32)
    lk2 = np.asarray(lambda_k2, dtype=np.float32)
    lam_init = np.float32(np.asarray(lambda_init).reshape(()))
    gnw = np.asarray(gn_weight, dtype=np.float32)
    gnb = np.asarray(gn_bias, dtype=np.float32)

    nc, (cosT, ssinT) = _get_module()

    in_maps = []
    for core in range(8):
        b = core // 4
        hb = (core % 4) * HPC
        c1 = slice(hb * Hd, (hb + HPC) * Hd)
        c2 = slice(H * Hd + hb * Hd, H * Hd + (hb + HPC) * Hd)
        lam = np.stack([lq1[hb:hb + HPC], lk1[hb:hb + HPC],
                        lq2[hb:hb + HPC], lk2[hb:hb + HPC],
                        np.full(HPC, lam_init, np.float32)]).astype(np.float32)
        in_maps.append({
            "x": np.ascontiguousarray(x[b]),
            "wq": np.ascontiguousarray(np.concatenate([Wq[:, c1], Wq[:, c2]], axis=1)),
            "wk": np.ascontiguousarray(np.concatenate([Wk[:, c1], Wk[:, c2]], axis=1)),
            "wv": np.ascontiguousarray(Wv[:, c1]),
            "wo": np.ascontiguousarray(Wo[c1, :]),
            "lam": lam,
            "gnw": np.ascontiguousarray(gnw[c1]),
            "gnb": np.ascontiguousarray(gnb[c1]),
            "cosT": cosT,
            "ssinT": ssinT,
        })

    last_err = None
    for attempt in range(3):
        try:
            res = run_bass_kernel_spmd(nc, in_maps, core_ids=list(range(8)))
            break
        except Exception as e:  # transient axon/device hiccups
            last_err = e
            time.sleep(10 * (attempt + 1))
    else:
        raise last_err

    out = np.zeros((B, S, D), dtype=np.float32)
    for core in range(8):
        out[core // 4] += res.results[core]["y"]
    return out


# revision 8
# speedup vs baseline: 1.2475x; 1.1188x over previous
"""Differential attention (nn_DifferentialAttention_84679575208071) on 8 TRN2
NeuronCores via Bass/Tile.

Sharding: hybrid data-parallel x tensor-parallel. Core c handles batch c//4 and
heads 4*(c%4) .. 4*(c%4)+4 (B=2, H=16 -> 2 batch groups x 4-way head split).
Each core computes its 4 heads' attention + group-norm + partial o_proj; the
host sums the 4 partial (S, D) outputs per batch (tensor-parallel o_proj
reduction) and stacks the 2 batches.

Per-core algorithm (all matmuls f32r = full-rate PE):
  xT = x^T (PE transposes)                        [D, S]
  qT/kT = (Wq/Wk slice)^T x^T with RoPE applied   [512, S] (2 branches x 4 heads x 64)
  v    = x Wv slice (row layout)                  [S, 256], augmented with a
         ones column per head -> PV matmul also yields softmax denominators
  attention, chunk-major: per (sq-chunk, head): both branches' score tiles go
         into one [128,1024] PSUM -> one wide exp (no max subtraction; scores
         are O(+-6) so fp32 exp is exact-safe) -> two PV matmuls [V|1]^T E^T
  diff = O1 - (lam*r1/r2) * O2  (group-norm is scale-invariant up to eps, so
         normalizing by r1 is folded in; eps is scaled by r1^2 to compensate)
  group-norm over each head's 64 channels, * gn_weight*(1-lam_init) + bias,
         then o_proj — both streamed per chunk so they overlap the next
         chunk's attention
"""
import json
import time

import numpy as np

import bass_rust
import concourse.bass as bass
import concourse.mybir as mybir
import concourse.tile as tile

F32 = mybir.dt.float32
F32R = mybir.dt.float32r
MULT = mybir.AluOpType.mult
ADD = mybir.AluOpType.add
SUB = mybir.AluOpType.subtract
AX = mybir.AxisListType.X
AF = mybir.ActivationFunctionType

B, S, D = 2, 2048, 1024
H, Hd = 16, 64
HPC = 4               # heads per core
THDC = HPC * Hd       # 256 channels per core
NT = S // 128         # 16 s-tiles
NCH = S // 512        # 4 chunks
DK = D // 128         # 8 contraction tiles
LAMBDA_INIT = 0.8
GN_EPS = 1e-5
ROPE_BASE = 10000.0
QK_SCALE = float(Hd) ** -0.5  # 0.125


def _split_multi_waits(nc):
    """This container's walrus rejects >1 sync wait per instruction. Hoist
    extra waits onto same-engine NoOps inserted right before the instruction
    (engine queues are in-order, so semantics are unchanged)."""
    d = json.loads(bass_rust.module_to_json_string(nc.m))
    ctr = 0
    for f in d["functions"]:
        for bb in f["blocks"]:
            out = []
            for inst in bb.get("instructions", []):
                si = inst.get("sync_info")
                waits = si.get("on_wait", []) if si else []
                if len(waits) > 1:
                    for w in waits[:-1]:
                        ctr += 1
                        out.append({
                            "debug": inst.get("debug", 0),
                            "engine": inst["engine"],
                            "ins": [],
                            "name": f"WSPLIT-{ctr}",
                            "opcode": "NoOp",
                            "outs": [],
                            "sync_info": {"on_update": [], "on_wait": [w]},
                        })
                    si["on_wait"] = [waits[-1]]
                out.append(inst)
            bb["instructions"] = out
    nc.m = bass_rust.module_from_json_string(json.dumps(d))


def _rope_tables():
    inv_freq = 1.0 / (ROPE_BASE ** (np.arange(0, Hd, 2, dtype=np.float32) / Hd))
    t = np.arange(S, dtype=np.float32)
    freqs = np.outer(t, inv_freq).astype(np.float32)
    emb = np.concatenate([freqs, freqs], axis=-1)       # (S, Hd)
    cos = np.cos(emb).astype(np.float32)
    sin = np.sin(emb).astype(np.float32)
    # sign-folded shifted-sin table: rotate_half contributes
    # out[d] = q[(d+32)%64] * g[d], g = [-sin[:32] | +sin[32:]]
    g = np.concatenate([-sin[:, :32], sin[:, 32:]], axis=1)
    return cos.T.copy(), g.T.copy()                     # (Hd, S) each


def _hview(t):
    """[128, 260] tile -> [128, 4, 64] per-head data columns."""
    return t[:, :].rearrange("p (h c) -> p h c", h=HPC)[:, :, 0:Hd]


def _rview(t):
    """[128, 260] tile -> [128, 4] per-head r (denominator) columns."""
    v = t[:, :].rearrange("p (h c) -> p h c", h=HPC)[:, :, Hd:Hd + 1]
    return v.rearrange("p h c -> p (h c)")


def _b64(ap):
    """[128, 4] -> [128, 4, 64] broadcast along a new inner dim."""
    return ap.unsqueeze(2).broadcast_to((128, HPC, Hd))


def build_module():
    nc = bass.Bass(trn_type="TRN2")

    x_d = nc.dram_tensor("x", [S, D], F32, kind="ExternalInput")
    wq_d = nc.dram_tensor("wq", [D, 512], F32, kind="ExternalInput")
    wk_d = nc.dram_tensor("wk", [D, 512], F32, kind="ExternalInput")
    wv_d = nc.dram_tensor("wv", [D, THDC], F32, kind="ExternalInput")
    wo_d = nc.dram_tensor("wo", [THDC, D], F32, kind="ExternalInput")
    lam_d = nc.dram_tensor("lam", [5, HPC], F32, kind="ExternalInput")
    gnw_d = nc.dram_tensor("gnw", [THDC], F32, kind="ExternalInput")
    gnb_d = nc.dram_tensor("gnb", [THDC], F32, kind="ExternalInput")
    cos_d = nc.dram_tensor("cosT", [Hd, S], F32, kind="ExternalInput")
    sin_d = nc.dram_tensor("ssinT", [Hd, S], F32, kind="ExternalInput")
    y_d = nc.dram_tensor("y", [S, D], F32, kind="ExternalOutput")
    ident_d = nc.inline_tensor(np.eye(128, dtype=np.float32), name="identity")

    dma = nc.sync.dma_start  # HWDGE: no Pool-engine descriptor-prep cost

    with tile.TileContext(nc) as tc:
        with tc.tile_pool(name="persist", bufs=1) as pA, \
             tc.tile_pool(name="dscr", bufs=1, space="DRAM") as pD:
            qt = [pA.tile([128, S], F32R, tag=f"qt{i}", name=f"qt{i}") for i in range(4)]
            kt = [pA.tile([128, S], F32R, tag=f"kt{i}", name=f"kt{i}") for i in range(4)]
            vaug = [pA.tile([128, HPC * (Hd + 1)], F32R, tag=f"va{i}", name=f"va{i}")
                    for i in range(NT)]
            ident = pA.tile([128, 128], F32, tag="ident", name="ident")
            dma(out=ident, in_=ident_d.ap())

            ones1 = pA.tile([128, 1], F32, tag="ones1", name="ones1")
            nc.vector.memset(ones1, 1.0)

            # ---- phases 1-2: x^T, V/K/Q projections, rope ----
            with tc.tile_pool(name="ph12", bufs=1) as pB, \
                 tc.tile_pool(name="xload", bufs=2) as pX, \
                 tc.tile_pool(name="wraw", bufs=3) as pWr, \
                 tc.tile_pool(name="wrow", bufs=8) as pW, \
                 tc.tile_pool(name="wvrow", bufs=8) as pWv, \
                 tc.tile_pool(name="trot", bufs=3) as pT, \
                 tc.tile_pool(name="ps_tr", bufs=4, space="PSUM") as pPtr, \
                 tc.tile_pool(name="ps_proj", bufs=3, space="PSUM") as pPproj:
                cos2h = pB.tile([128, S], F32, tag="cos2h", name="cos2h")
                sin2h = pB.tile([128, S], F32, tag="sin2h", name="sin2h")
                xT = [pB.tile([128, S // 2], F32R, tag=f"xT{i}", name=f"xT{i}")
                      for i in range(DK)]

                def rope(dst, tt_i, c, psq):
                    # qsh = rotate_half permutation of psq, staged via shifted
                    # ACT copies (PSUM reads would force DVE 1x mode); then
                    # both DVE muls run 2x from SBUF.
                    cs = slice(c * 512, (c + 1) * 512)
                    qsh = pT.tile([128, 512], F32, tag="qsh", name=f"qsh{tt_i}_{c}")
                    for hb in (0, 64):
                        nc.scalar.copy(out=qsh[hb:hb + 32, :], in_=psq[hb + 32:hb + 64, :])
                        nc.scalar.copy(out=qsh[hb + 32:hb + 64, :], in_=psq[hb:hb + 32, :])
                    nc.vector.tensor_tensor(out=dst[tt_i][:, cs], in0=psq,
                                            in1=cos2h[:, cs], op=MULT)
                    trot = pT.tile([128, 512], F32, tag="trot", name=f"trot{tt_i}_{c}")
                    nc.vector.tensor_tensor(out=trot, in0=qsh, in1=sin2h[:, cs], op=MULT)
                    nc.vector.tensor_tensor(out=dst[tt_i][:, cs], in0=dst[tt_i][:, cs],
                                            in1=trot, op=ADD)

                for sg in range(2):
                    # x^T for this half (x loads split for earlier first transpose)
                    for stl in range(8):
                        st = sg * 8 + stl
                        xtile = pX.tile([128, D], F32, tag="xtile", name=f"xt{st}")
                        dma(out=xtile[:, 0:512], in_=x_d[st * 128:(st + 1) * 128, 0:512])
                        dma(out=xtile[:, 512:1024], in_=x_d[st * 128:(st + 1) * 128, 512:1024])
                        for dk in range(DK):
                            ptr = pPtr.tile([128, 128], F32, tag="ptr", name=f"ptx{st}_{dk}")
                            nc.tensor.transpose(ptr, xtile[:, dk * 128:(dk + 1) * 128], ident)
                            nc.scalar.copy(out=xT[dk][:, stl * 128:(stl + 1) * 128], in_=ptr)

                    if sg == 0:
                        # rope tables: emitted after x loads so they don't
                        # delay the critical first transposes
                        dma(out=cos2h[0:64, :], in_=cos_d[:, :])
                        dma(out=cos2h[64:128, :], in_=cos_d[:, :])
                        dma(out=sin2h[0:64, :], in_=sin_d[:, :])
                        dma(out=sin2h[64:128, :], in_=sin_d[:, :])

                    # V projection first (attention is gated on it)
                    wvrow = []
                    for dk in range(DK):
                        raw = pWr.tile([128, THDC], F32, tag="wvraw", name=f"wvr{sg}{dk}")
                        dma(out=raw, in_=wv_d[dk * 128:(dk + 1) * 128, :])
                        wr = pWv.tile([128, THDC], F32R, tag="wvrow", name=f"wv{sg}{dk}")
                        nc.vector.tensor_copy(out=wr, in_=raw)
                        wvrow.append(wr)
                    for stl in range(8):
                        st = sg * 8 + stl
                        psv = pPproj.tile([128, THDC], F32, tag="psq", name=f"psv{st}")
                        for i in range(DK):
                            dk = (i + stl) % DK
                            nc.tensor.matmul(psv, lhsT=xT[dk][:, stl * 128:(stl + 1) * 128],
                                             rhs=wvrow[dk], start=(i == 0), stop=(i == DK - 1))
                        va = vaug[st]
                        vav = va[:, :].rearrange("p (h c) -> p h c", h=HPC)
                        nc.vector.tensor_copy(out=vav[:, :, 0:Hd],
                                              in_=psv[:, :].rearrange("p (h c) -> p h c", h=HPC))
                        for h in range(HPC):
                            nc.vector.tensor_copy(out=va[:, h * 65 + 64:h * 65 + 65], in_=ones1)

                    # K then Q projections + rope; heads 0/1 tiles (tt 0,2) first
                    for which, (w_dram, dst) in enumerate(((wk_d, kt), (wq_d, qt))):
                        wrow = []
                        for dk in range(DK):
                            raw = pWr.tile([128, 512], F32, tag="wraw", name=f"wr{sg}{which}{dk}")
                            dma(out=raw, in_=w_dram[dk * 128:(dk + 1) * 128, :])
                            wr = pW.tile([128, 512], F32R, tag="wrow", name=f"w{sg}{which}{dk}")
                            nc.vector.tensor_copy(out=wr, in_=raw)
                            wrow.append(wr)
                        for tt_i in (0, 2, 1, 3):
                            for cl in range(2):
                                c = sg * 2 + cl
                                psq = pPproj.tile([128, 512], F32, tag="psq",
                                                  name=f"psq{which}{tt_i}_{c}")
                                for i in range(DK):
                                    dk = (i + tt_i * 2 + cl) % DK
                                    nc.tensor.matmul(
                                        psq, lhsT=wrow[dk][:, tt_i * 128:(tt_i + 1) * 128],
                                        rhs=xT[dk][:, cl * 512:(cl + 1) * 512],
                                        start=(i == 0), stop=(i == DK - 1))
                                rope(dst, tt_i, c, psq)

            # ---- phase 0: lambdas, gn scale/bias prep ----
            lamsb = pA.tile([1, 20], F32, tag="lamsb", name="lamsb")
            dma(out=lamsb, in_=lam_d[:, :].rearrange("a b -> (a b)").unsqueeze(0))
            lt = pA.tile([1, 8], F32, tag="lt", name="lt")
            nc.vector.tensor_tensor(out=lt[:, 0:4], in0=lamsb[:, 0:4], in1=lamsb[:, 4:8], op=MULT)
            nc.vector.tensor_tensor(out=lt[:, 4:8], in0=lamsb[:, 8:12], in1=lamsb[:, 12:16], op=MULT)
            nc.scalar.activation(out=lt, in_=lt, func=AF.Exp)
            lamv = pA.tile([1, 4], F32, tag="lamv", name="lamv")
            nc.vector.tensor_tensor(out=lamv, in0=lt[:, 0:4], in1=lt[:, 4:8], op=SUB)
            nc.vector.tensor_tensor(out=lamv, in0=lamv, in1=lamsb[:, 16:20], op=ADD)
            slam = pA.tile([1, 1], F32, tag="slam", name="slam")
            nc.scalar.activation(out=slam, in_=lamsb[:, 16:17], func=AF.Identity,
                                 scale=-1.0, bias=1.0)
            dscr = pD.tile([1, 8], F32, name="dscr")
            dma(out=dscr[0:1, 0:4], in_=lamv)
            dma(out=dscr[0:1, 4:5], in_=slam)
            lam_b = pA.tile([128, 4], F32, tag="lam_b", name="lam_b")
            dma(out=lam_b, in_=dscr[0:1, 0:4].squeeze(0).partition_broadcast(128))
            slam_b = pA.tile([128, 1], F32, tag="slam_b", name="slam_b")
            dma(out=slam_b, in_=dscr[0:1, 4:5].squeeze(0).partition_broadcast(128))
            gnw_eff = pA.tile([128, THDC], F32, tag="gnw_eff", name="gnw_eff")
            dma(out=gnw_eff, in_=gnw_d.ap().partition_broadcast(128))
            nc.vector.tensor_scalar_mul(gnw_eff, gnw_eff, slam_b[:, 0:1])
            gnb_eff = pA.tile([128, THDC], F32, tag="gnb_eff", name="gnb_eff")
            dma(out=gnb_eff, in_=gnb_d.ap().partition_broadcast(128))
            nc.vector.tensor_scalar_mul(gnb_eff, gnb_eff, slam_b[:, 0:1])
            # ---- phases 3-5: attention + group norm + o_proj, chunk-major ----
            with tc.tile_pool(name="owyt", bufs=1) as pC0, \
                 tc.tile_pool(name="etile", bufs=4) as pE, \
                 tc.tile_pool(name="osb", bufs=4) as pO, \
                 tc.tile_pool(name="gn", bufs=2) as pG, \
                 tc.tile_pool(name="outsb", bufs=3) as pOut, \
                 tc.tile_pool(name="wosb", bufs=1) as pWo, \
                 tc.tile_pool(name="ps_s", bufs=3, space="PSUM") as pPs, \
                 tc.tile_pool(name="ps_o", bufs=2, space="PSUM") as pPo, \
                 tc.tile_pool(name="ps_out", bufs=1, space="PSUM") as pPout, \
                 tc.tile_pool(name="ps_t2", bufs=2, space="PSUM") as pPt2:
                ow = [[pC0.tile([128, HPC * (Hd + 1)], F32, tag=f"ow{br}_{i}",
                                name=f"ow{br}_{i}") for i in range(NT)] for br in range(2)]
                yt = [pC0.tile([128, S], F32R, tag=f"yt{i}", name=f"yt{i}")
                      for i in range(2)]
                wo_sb = []
                for ci in range(2):
                    raw = pWo.tile([128, D], F32, tag=f"woraw{ci}", name=f"woraw{ci}")
                    dma(out=raw, in_=wo_d[ci * 128:(ci + 1) * 128, :])
                    wr = pWo.tile([128, D], F32R, tag=f"wo{ci}", name=f"wo{ci}")
                    nc.vector.tensor_copy(out=wr, in_=raw)
                    wo_sb.append(wr)

                for c in range(NCH):
                    cs = slice(c * 512, (c + 1) * 512)
                    for h in range(HPC):
                        tt1 = h // 2
                        tt2 = 2 + h // 2
                        ro = (h % 2) * 64
                        pso = [pPo.tile([65, 512], F32, tag="pso", name=f"pso{c}{h}{br}")
                               for br in range(2)]
                        for sk in range(NT):
                            va = vaug[sk][:, h * 65:(h + 1) * 65]
                            for br, tt_i in ((0, tt1), (1, tt2)):
                                pss = pPs.tile([128, 512], F32, tag="pss",
                                               name=f"pss{c}{h}{sk}{br}")
                                nc.tensor.matmul(pss,
                                                 lhsT=kt[tt_i][ro:ro + 64, sk * 128:(sk + 1) * 128],
                                                 rhs=qt[tt_i][ro:ro + 64, cs],
                                                 start=True, stop=True)
                                e = pE.tile([128, 512], F32R, tag="e", name=f"e{c}{h}{sk}{br}")
                                nc.scalar.activation(out=e, in_=pss, func=AF.Exp,
                                                     scale=QK_SCALE)
                                nc.tensor.matmul(pso[br], lhsT=va, rhs=e,
                                                 start=(sk == 0), stop=(sk == NT - 1))
                        for br in range(2):
                            osb = pO.tile([65, 512], F32, tag="osb", name=f"osb{c}{h}{br}")
                            nc.vector.tensor_copy(out=osb, in_=pso[br])
                            for stl in range(4):
                                st = c * 4 + stl
                                ptr = pPt2.tile([128, 65], F32, tag="ptr2",
                                                name=f"po{c}{h}{br}{stl}")
                                nc.tensor.transpose(ptr, osb[:, stl * 128:(stl + 1) * 128],
                                                    ident[0:65, 0:65])
                                nc.vector.tensor_copy(out=ow[br][st][:, h * 65:(h + 1) * 65],
                                                      in_=ptr)

                    # group norm + y^T + o_proj for this chunk's 4 s-tiles
                    for stl in range(4):
                        st = c * 4 + stl
                        ow1, ow2 = ow[0][st], ow[1][st]
                        rec = pG.tile([128, 4], F32, tag="rec", name=f"rec{st}")
                        nc.vector.reciprocal(rec, _rview(ow2))
                        rho = pG.tile([128, 4], F32, tag="rho", name=f"rho{st}")
                        nc.vector.tensor_tensor(out=rho, in0=_rview(ow1), in1=rec, op=MULT)
                        nc.vector.tensor_tensor(out=rho, in0=rho, in1=lam_b, op=MULT)
                        dt_ = pG.tile([128, THDC], F32, tag="dt", name=f"dt{st}")
                        dtv = dt_[:, :].rearrange("p (h c) -> p h c", h=HPC)
                        nc.vector.tensor_tensor(out=dtv, in0=_hview(ow2), in1=_b64(rho), op=MULT)
                        nc.vector.tensor_tensor(out=dtv, in0=_hview(ow1), in1=dtv, op=SUB)
                        s1 = pG.tile([128, 4], F32, tag="s1", name=f"s1{st}")
                        nc.vector.reduce_sum(out=s1, in_=dtv, axis=AX)
                        nc.vector.tensor_scalar_mul(s1, s1, -1.0 / Hd)
                        nc.vector.tensor_tensor(out=dtv, in0=dtv, in1=_b64(s1), op=ADD)
                        d2 = pG.tile([128, THDC], F32, tag="d2", name=f"d2{st}")
                        nc.gpsimd.tensor_tensor(out=d2, in0=dt_, in1=dt_, op=MULT)
                        s2 = pG.tile([128, 4], F32, tag="s2", name=f"s2{st}")
                        nc.vector.reduce_sum(out=s2,
                                             in_=d2[:, :].rearrange("p (h c) -> p h c", h=HPC),
                                             axis=AX)
                        nc.vector.tensor_tensor(out=rec, in0=_rview(ow1), in1=_rview(ow1), op=MULT)
                        nc.vector.tensor_scalar_mul(rec, rec, GN_EPS)
                        nc.vector.tensor_scalar_mul(s2, s2, 1.0 / Hd)
                        nc.vector.tensor_tensor(out=s2, in0=s2, in1=rec, op=ADD)
                        nc.scalar.activation(out=s2, in_=s2, func=AF.Sqrt)
                        nc.vector.reciprocal(s2, s2)
                        nc.vector.tensor_tensor(out=dtv, in0=dtv, in1=_b64(s2), op=MULT)
                        nc.gpsimd.tensor_tensor(out=dt_, in0=dt_, in1=gnw_eff, op=MULT)
                        nc.gpsimd.tensor_tensor(out=dt_, in0=dt_, in1=gnb_eff, op=ADD)
                        for ci in range(2):
                            ptr = pPt2.tile([128, 128], F32, tag="ptr2", name=f"py{st}_{ci}")
                            nc.tensor.transpose(ptr, dt_[:, ci * 128:(ci + 1) * 128], ident)
                            nc.vector.tensor_copy(out=yt[ci][:, st * 128:(st + 1) * 128],
                                                  in_=ptr)
                        for oc in range(2):
                            pout = pPout.tile([128, 512], F32, tag="pout", name=f"pout{st}_{oc}")
                            for ci in range(2):
                                nc.tensor.matmul(pout,
                                                 lhsT=yt[ci][:, st * 128:(st + 1) * 128],
                                                 rhs=wo_sb[ci][:, oc * 512:(oc + 1) * 512],
                                                 start=(ci == 0), stop=(ci == 1))
                            ost = pOut.tile([128, 512], F32, tag="ost", name=f"ost{st}_{oc}")
                            nc.vector.tensor_copy(out=ost, in_=pout)
                            dma(out=y_d[st * 128:(st + 1) * 128, oc * 512:(oc + 1) * 512],
                                in_=ost)

    _split_multi_waits(nc)
    return nc


_CACHE = {}


def _get_module():
    if "nc" not in _CACHE:
        _CACHE["nc"] = build_module()
        _CACHE["tables"] = _rope_tables()
    return _CACHE["nc"], _CACHE["tables"]


def kernel(x, Wq, Wk, Wv, Wo, lambda_q1, lambda_k1, lambda_q2, lambda_k2,
           lambda_init, gn_weight, gn_bias):
    from concourse.bass_utils import run_bass_kernel_spmd

    x = np.ascontiguousarray(np.asarray(x, dtype=np.float32))
    Wq = np.asarray(Wq, dtype=np.float32)
    Wk = np.asarray(Wk, dtype=np.float32)
    Wv = np.asarray(Wv, dtype=np.float32)
    Wo = np.asarray(Wo, dtype=np.float32)
    lq1 = np.asarray(lambda_q1, dtype=np.float32)
    lk1 = np.asarray(lambda_k1, dtype=np.float32)
    lq2 = np.asarray(lambda_q2, dtype=np.float32)
    lk2 = np.asarray(lambda_k2, dtype=np.float32)
    lam_init = np.float32(np.asarray(lambda_init).reshape(()))
    gnw = np.asarray(gn_weight, dtype=np.float32)
    gnb = np.asarray(gn_bias, dtype=np.float32)

    nc, (cosT, ssinT) = _get_module()

    in_maps = []
    for core in range(8):
        b = core // 4
        hb = (core % 4) * HPC
        c1 = slice(hb * Hd, (hb + HPC) * Hd)
        c2 = slice(H * Hd + hb * Hd, H * Hd + (hb + HPC) * Hd)
        lam = np.stack([lq1[hb:hb + HPC], lk1[hb:hb + HPC],
                        lq2[hb:hb + HPC], lk2[hb:hb + HPC],
                        np.full(HPC, lam_init, np.float32)]).astype(np.float32)
        in_maps.append({
            "x": np.ascontiguousarray(x[b]),
            "wq": np.ascontiguousarray(np.concatenate([Wq[:, c1], Wq[:, c2]], axis=1)),
            "wk": np.ascontiguousarray(np.concatenate([Wk[:, c1], Wk[:, c2]], axis=1)),
            "wv": np.ascontiguousarray(Wv[:, c1]),
            "wo": np.ascontiguousarray(Wo[c1, :]),
            "lam": lam,
            "gnw": np.ascontiguousarray(gnw[c1]),
            "gnb": np.ascontiguousarray(gnb[c1]),
            "cosT": cosT,
            "ssinT": ssinT,
        })

    last_err = None
    for attempt in range(3):
        try:
            res = run_bass_kernel_spmd(nc, in_maps, core_ids=list(range(8)))
            break
        except Exception as e:  # transient axon/device hiccups
            last_err = e
            time.sleep(10 * (attempt + 1))
    else:
        raise last_err

    out = np.zeros((B, S, D), dtype=np.float32)
    for core in range(8):
        out[core // 4] += res.results[core]["y"]
    return out


# revision 9
# speedup vs baseline: 1.3044x; 1.0456x over previous
"""Differential attention (nn_DifferentialAttention_84679575208071) on 8 TRN2
NeuronCores via Bass/Tile.

Sharding: hybrid data-parallel x tensor-parallel. Core c handles batch c//4 and
heads 4*(c%4) .. 4*(c%4)+4 (B=2, H=16 -> 2 batch groups x 4-way head split).
Each core computes its 4 heads' attention + group-norm + partial o_proj; the
host sums the 4 partial (S, D) outputs per batch (tensor-parallel o_proj
reduction) and stacks the 2 batches.

Per-core algorithm (all matmuls f32r = full-rate PE):
  xT = x^T (PE transposes)                        [D, S]
  qT/kT = (Wq/Wk slice)^T x^T with RoPE applied   [512, S] (2 branches x 4 heads x 64)
  v    = x Wv slice (row layout)                  [S, 256], augmented with a
         ones column per head -> PV matmul also yields softmax denominators
  attention, chunk-major: per (sq-chunk, head): both branches' score tiles go
         into one [128,1024] PSUM -> one wide exp (no max subtraction; scores
         are O(+-6) so fp32 exp is exact-safe) -> two PV matmuls [V|1]^T E^T
  diff = O1 - (lam*r1/r2) * O2  (group-norm is scale-invariant up to eps, so
         normalizing by r1 is folded in; eps is scaled by r1^2 to compensate)
  group-norm over each head's 64 channels, * gn_weight*(1-lam_init) + bias,
         then o_proj — both streamed per chunk so they overlap the next
         chunk's attention
"""
import json
import time

import numpy as np

import bass_rust
import concourse.bass as bass
import concourse.mybir as mybir
import concourse.tile as tile

F32 = mybir.dt.float32
F32R = mybir.dt.float32r
BF16 = mybir.dt.bfloat16
MULT = mybir.AluOpType.mult
ADD = mybir.AluOpType.add
SUB = mybir.AluOpType.subtract
AX = mybir.AxisListType.X
AF = mybir.ActivationFunctionType

B, S, D = 2, 2048, 1024
H, Hd = 16, 64
HPC = 4               # heads per core
THDC = HPC * Hd       # 256 channels per core
NT = S // 128         # 16 s-tiles
NCH = S // 512        # 4 chunks
DK = D // 128         # 8 contraction tiles
LAMBDA_INIT = 0.8
GN_EPS = 1e-5
ROPE_BASE = 10000.0
QK_SCALE = float(Hd) ** -0.5  # 0.125


def _split_multi_waits(nc):
    """This container's walrus rejects >1 sync wait per instruction. Hoist
    extra waits onto same-engine NoOps inserted right before the instruction
    (engine queues are in-order, so semantics are unchanged)."""
    d = json.loads(bass_rust.module_to_json_string(nc.m))
    ctr = 0
    for f in d["functions"]:
        for bb in f["blocks"]:
            out = []
            for inst in bb.get("instructions", []):
                si = inst.get("sync_info")
                waits = si.get("on_wait", []) if si else []
                if len(waits) > 1:
                    for w in waits[:-1]:
                        ctr += 1
                        out.append({
                            "debug": inst.get("debug", 0),
                            "engine": inst["engine"],
                            "ins": [],
                            "name": f"WSPLIT-{ctr}",
                            "opcode": "NoOp",
                            "outs": [],
                            "sync_info": {"on_update": [], "on_wait": [w]},
                        })
                    si["on_wait"] = [waits[-1]]
                out.append(inst)
            bb["instructions"] = out
    nc.m = bass_rust.module_from_json_string(json.dumps(d))


def _rope_tables():
    inv_freq = 1.0 / (ROPE_BASE ** (np.arange(0, Hd, 2, dtype=np.float32) / Hd))
    t = np.arange(S, dtype=np.float32)
    freqs = np.outer(t, inv_freq).astype(np.float32)
    emb = np.concatenate([freqs, freqs], axis=-1)       # (S, Hd)
    cos = np.cos(emb).astype(np.float32)
    sin = np.sin(emb).astype(np.float32)
    # sign-folded shifted-sin table: rotate_half contributes
    # out[d] = q[(d+32)%64] * g[d], g = [-sin[:32] | +sin[32:]]
    g = np.concatenate([-sin[:, :32], sin[:, 32:]], axis=1)
    return cos.T.copy(), g.T.copy()                     # (Hd, S) each


def _hview(t):
    """[128, 260] tile -> [128, 4, 64] per-head data columns."""
    return t[:, :].rearrange("p (h c) -> p h c", h=HPC)[:, :, 0:Hd]


def _rview(t):
    """[128, 260] tile -> [128, 4] per-head r (denominator) columns."""
    v = t[:, :].rearrange("p (h c) -> p h c", h=HPC)[:, :, Hd:Hd + 1]
    return v.rearrange("p h c -> p (h c)")


def _b64(ap):
    """[128, 4] -> [128, 4, 64] broadcast along a new inner dim."""
    return ap.unsqueeze(2).broadcast_to((128, HPC, Hd))


def build_module():
    nc = bass.Bass(trn_type="TRN2")

    x_d = nc.dram_tensor("x", [S, D], F32, kind="ExternalInput")
    wq_d = nc.dram_tensor("wq", [D, 512], F32, kind="ExternalInput")
    wk_d = nc.dram_tensor("wk", [D, 512], F32, kind="ExternalInput")
    wv_d = nc.dram_tensor("wv", [D, THDC], F32, kind="ExternalInput")
    wo_d = nc.dram_tensor("wo", [THDC, D], F32, kind="ExternalInput")
    lam_d = nc.dram_tensor("lam", [5, HPC], F32, kind="ExternalInput")
    gnw_d = nc.dram_tensor("gnw", [THDC], F32, kind="ExternalInput")
    gnb_d = nc.dram_tensor("gnb", [THDC], F32, kind="ExternalInput")
    cos_d = nc.dram_tensor("cosT", [Hd, S], F32, kind="ExternalInput")
    sin_d = nc.dram_tensor("ssinT", [Hd, S], F32, kind="ExternalInput")
    y_d = nc.dram_tensor("y", [S, D], F32, kind="ExternalOutput")
    ident_d = nc.inline_tensor(np.eye(128, dtype=np.float32), name="identity")

    dma = nc.sync.dma_start  # HWDGE: no Pool-engine descriptor-prep cost

    with tile.TileContext(nc) as tc:
        with tc.tile_pool(name="persist", bufs=1) as pA, \
             tc.tile_pool(name="dscr", bufs=1, space="DRAM") as pD:
            qt = [pA.tile([128, S], BF16, tag=f"qt{i}", name=f"qt{i}") for i in range(4)]
            kt = [pA.tile([128, S], BF16, tag=f"kt{i}", name=f"kt{i}") for i in range(4)]
            vaug = [pA.tile([128, HPC * (Hd + 1)], F32R, tag=f"va{i}", name=f"va{i}")
                    for i in range(NT)]
            ident = pA.tile([128, 128], F32, tag="ident", name="ident")
            dma(out=ident, in_=ident_d.ap())

            ones1 = pA.tile([128, 1], F32, tag="ones1", name="ones1")
            nc.vector.memset(ones1, 1.0)

            # ---- phases 1-2: x^T (full S), V/K/Q projections, rope ----
            with tc.tile_pool(name="ph12", bufs=1) as pB, \
                 tc.tile_pool(name="xload", bufs=4) as pX, \
                 tc.tile_pool(name="wraw", bufs=4) as pWr, \
                 tc.tile_pool(name="wtile", bufs=10) as pW, \
                 tc.tile_pool(name="wvrow", bufs=8) as pWv, \
                 tc.tile_pool(name="qsh", bufs=2) as pQs, \
                 tc.tile_pool(name="trot", bufs=2) as pT, \
                 tc.tile_pool(name="ps_tr", bufs=4, space="PSUM") as pPtr, \
                 tc.tile_pool(name="ps_proj", bufs=3, space="PSUM") as pPproj:
                cos2h = pB.tile([128, S], F32, tag="cos2h", name="cos2h")
                sin2h = pB.tile([128, S], F32, tag="sin2h", name="sin2h")
                xT = [pB.tile([128, S], F32R, tag=f"xT{i}", name=f"xT{i}")
                      for i in range(DK)]

                def rope(dst, tt_i, c, psq):
                    # qsh = rotate_half permutation of psq, staged via shifted
                    # ACT copies (PSUM reads would force DVE 1x mode); the two
                    # DVE muls then run 2x from SBUF.
                    cs = slice(c * 512, (c + 1) * 512)
                    qsh = pQs.tile([128, 512], F32, tag="qsh", name=f"qsh{tt_i}_{c}")
                    for hb in (0, 64):
                        nc.scalar.copy(out=qsh[hb:hb + 32, :], in_=psq[hb + 32:hb + 64, :])
                        nc.scalar.copy(out=qsh[hb + 32:hb + 64, :], in_=psq[hb:hb + 32, :])
                    nc.vector.tensor_tensor(out=dst[tt_i][:, cs], in0=psq,
                                            in1=cos2h[:, cs], op=MULT)
                    trot = pT.tile([128, 512], F32, tag="trot", name=f"trot{tt_i}_{c}")
                    nc.vector.tensor_tensor(out=trot, in0=qsh, in1=sin2h[:, cs], op=MULT)
                    nc.vector.tensor_tensor(out=dst[tt_i][:, cs], in0=dst[tt_i][:, cs],
                                            in1=trot, op=ADD)

                # x^T: per s-tile, two half loads + 8 transposes
                for st in range(NT):
                    halves = []
                    for hf in range(2):
                        xtile = pX.tile([128, 512], F32, tag="xtile", name=f"xt{st}_{hf}")
                        dma(out=xtile, in_=x_d[st * 128:(st + 1) * 128,
                                              hf * 512:(hf + 1) * 512])
                        halves.append(xtile)
                    if st == 1:
                        # rope tables: emitted after the first x loads so they
                        # don't delay the critical first transposes
                        dma(out=cos2h[0:64, :], in_=cos_d[:, :])
                        dma(out=cos2h[64:128, :], in_=cos_d[:, :])
                        dma(out=sin2h[0:64, :], in_=sin_d[:, :])
                        dma(out=sin2h[64:128, :], in_=sin_d[:, :])
                    for dk in range(DK):
                        ptr = pPtr.tile([128, 128], F32, tag="ptr", name=f"ptx{st}_{dk}")
                        nc.tensor.transpose(ptr, halves[dk // 4][:, (dk % 4) * 128:
                                                                  (dk % 4 + 1) * 128], ident)
                        nc.scalar.copy(out=xT[dk][:, st * 128:(st + 1) * 128], in_=ptr)

                # V projection first (attention is gated on it)
                wvrow = []
                for dk in range(DK):
                    raw = pWr.tile([128, THDC], F32, tag="wvraw", name=f"wvr{dk}")
                    dma(out=raw, in_=wv_d[dk * 128:(dk + 1) * 128, :])
                    wr = pWv.tile([128, THDC], F32R, tag="wvrow", name=f"wv{dk}")
                    nc.vector.tensor_copy(out=wr, in_=raw)
                    wvrow.append(wr)
                for st in range(NT):
                    psv = pPproj.tile([128, THDC], F32, tag="psq", name=f"psv{st}")
                    for i in range(DK):
                        dk = (i + st) % DK
                        nc.tensor.matmul(psv, lhsT=xT[dk][:, st * 128:(st + 1) * 128],
                                         rhs=wvrow[dk], start=(i == 0), stop=(i == DK - 1))
                    va = vaug[st]
                    vav = va[:, :].rearrange("p (h c) -> p h c", h=HPC)
                    nc.vector.tensor_copy(out=vav[:, :, 0:Hd],
                                          in_=psv[:, :].rearrange("p (h c) -> p h c", h=HPC))
                    for h in range(HPC):
                        nc.vector.tensor_copy(out=va[:, h * 65 + 64:h * 65 + 65], in_=ones1)

                # K then Q projections + rope; heads 0/1 tiles (tt 0,2) first
                for which, (w_dram, dst) in enumerate(((wk_d, kt), (wq_d, qt))):
                    for tt_i in (0, 2, 1, 3):
                        wtile = []
                        for dk in range(DK):
                            raw = pWr.tile([128, 128], F32, tag="wraw",
                                           name=f"wr{which}{tt_i}{dk}")
                            dma(out=raw, in_=w_dram[dk * 128:(dk + 1) * 128,
                                                    tt_i * 128:(tt_i + 1) * 128])
                            wr = pW.tile([128, 128], F32R, tag="wtile",
                                         name=f"w{which}{tt_i}{dk}")
                            nc.vector.tensor_copy(out=wr, in_=raw)
                            wtile.append(wr)
                        for c in range(NCH):
                            psq = pPproj.tile([128, 512], F32, tag="psq",
                                              name=f"psq{which}{tt_i}_{c}")
                            for i in range(DK):
                                dk = (i + tt_i * 2 + c) % DK
                                nc.tensor.matmul(
                                    psq, lhsT=wtile[dk],
                                    rhs=xT[dk][:, c * 512:(c + 1) * 512],
                                    start=(i == 0), stop=(i == DK - 1))
                            rope(dst, tt_i, c, psq)

            # ---- phase 0: lambdas, gn scale/bias prep ----
            lamsb = pA.tile([1, 20], F32, tag="lamsb", name="lamsb")
            dma(out=lamsb, in_=lam_d[:, :].rearrange("a b -> (a b)").unsqueeze(0))
            lt = pA.tile([1, 8], F32, tag="lt", name="lt")
            nc.vector.tensor_tensor(out=lt[:, 0:4], in0=lamsb[:, 0:4], in1=lamsb[:, 4:8], op=MULT)
            nc.vector.tensor_tensor(out=lt[:, 4:8], in0=lamsb[:, 8:12], in1=lamsb[:, 12:16], op=MULT)
            nc.scalar.activation(out=lt, in_=lt, func=AF.Exp)
            lamv = pA.tile([1, 4], F32, tag="lamv", name="lamv")
            nc.vector.tensor_tensor(out=lamv, in0=lt[:, 0:4], in1=lt[:, 4:8], op=SUB)
            nc.vector.tensor_tensor(out=lamv, in0=lamv, in1=lamsb[:, 16:20], op=ADD)
            slam = pA.tile([1, 1], F32, tag="slam", name="slam")
            nc.scalar.activation(out=slam, in_=lamsb[:, 16:17], func=AF.Identity,
                                 scale=-1.0, bias=1.0)
            dscr = pD.tile([1, 8], F32, name="dscr")
            dma(out=dscr[0:1, 0:4], in_=lamv)
            dma(out=dscr[0:1, 4:5], in_=slam)
            lam_b = pA.tile([128, 4], F32, tag="lam_b", name="lam_b")
            dma(out=lam_b, in_=dscr[0:1, 0:4].squeeze(0).partition_broadcast(128))
            slam_b = pA.tile([128, 1], F32, tag="slam_b", name="slam_b")
            dma(out=slam_b, in_=dscr[0:1, 4:5].squeeze(0).partition_broadcast(128))
            gnw_eff = pA.tile([128, THDC], F32, tag="gnw_eff", name="gnw_eff")
            dma(out=gnw_eff, in_=gnw_d.ap().partition_broadcast(128))
            nc.vector.tensor_scalar_mul(gnw_eff, gnw_eff, slam_b[:, 0:1])
            gnb_eff = pA.tile([128, THDC], F32, tag="gnb_eff", name="gnb_eff")
            dma(out=gnb_eff, in_=gnb_d.ap().partition_broadcast(128))
            nc.vector.tensor_scalar_mul(gnb_eff, gnb_eff, slam_b[:, 0:1])
            # ---- phases 3-5: attention + group norm + o_proj, chunk-major ----
            with tc.tile_pool(name="owyt", bufs=1) as pC0, \
                 tc.tile_pool(name="etile", bufs=4) as pE, \
                 tc.tile_pool(name="osb", bufs=4) as pO, \
                 tc.tile_pool(name="gn", bufs=2) as pG, \
                 tc.tile_pool(name="outsb", bufs=3) as pOut, \
                 tc.tile_pool(name="wosb", bufs=1) as pWo, \
                 tc.tile_pool(name="ps_s", bufs=3, space="PSUM") as pPs, \
                 tc.tile_pool(name="ps_o", bufs=2, space="PSUM") as pPo, \
                 tc.tile_pool(name="ps_out", bufs=1, space="PSUM") as pPout, \
                 tc.tile_pool(name="ps_t2", bufs=2, space="PSUM") as pPt2:
                ow = [[pC0.tile([128, HPC * (Hd + 1)], F32, tag=f"ow{br}_{i}",
                                name=f"ow{br}_{i}") for i in range(NT)] for br in range(2)]
                yt = [pC0.tile([128, S], F32R, tag=f"yt{i}", name=f"yt{i}")
                      for i in range(2)]
                wo_sb = []
                for ci in range(2):
                    raw = pWo.tile([128, D], F32, tag=f"woraw{ci}", name=f"woraw{ci}")
                    dma(out=raw, in_=wo_d[ci * 128:(ci + 1) * 128, :])
                    wr = pWo.tile([128, D], F32R, tag=f"wo{ci}", name=f"wo{ci}")
                    nc.vector.tensor_copy(out=wr, in_=raw)
                    wo_sb.append(wr)

                for c in range(NCH):
                    cs = slice(c * 512, (c + 1) * 512)
                    for h in range(HPC):
                        tt1 = h // 2
                        tt2 = 2 + h // 2
                        ro = (h % 2) * 64
                        pso = [pPo.tile([65, 512], F32, tag="pso", name=f"pso{c}{h}{br}")
                               for br in range(2)]
                        for sk in range(NT):
                            va = vaug[sk][:, h * 65:(h + 1) * 65]
                            for br, tt_i in ((0, tt1), (1, tt2)):
                                pss = pPs.tile([128, 512], F32, tag="pss",
                                               name=f"pss{c}{h}{sk}{br}")
                                nc.tensor.matmul(pss,
                                                 lhsT=kt[tt_i][ro:ro + 64, sk * 128:(sk + 1) * 128],
                                                 rhs=qt[tt_i][ro:ro + 64, cs],
                                                 start=True, stop=True)
                                e = pE.tile([128, 512], F32R, tag="e", name=f"e{c}{h}{sk}{br}")
                                nc.scalar.activation(out=e, in_=pss, func=AF.Exp,
                                                     scale=QK_SCALE)
                                nc.tensor.matmul(pso[br], lhsT=va, rhs=e,
                                                 start=(sk == 0), stop=(sk == NT - 1))
                        for br in range(2):
                            osb = pO.tile([65, 512], F32, tag="osb", name=f"osb{c}{h}{br}")
                            nc.vector.tensor_copy(out=osb, in_=pso[br])
                            for stl in range(4):
                                st = c * 4 + stl
                                ptr = pPt2.tile([128, 65], F32, tag="ptr2",
                                                name=f"po{c}{h}{br}{stl}")
                                nc.tensor.transpose(ptr, osb[:, stl * 128:(stl + 1) * 128],
                                                    ident[0:65, 0:65])
                                nc.vector.tensor_copy(out=ow[br][st][:, h * 65:(h + 1) * 65],
                                                      in_=ptr)

                    # group norm + y^T + o_proj for this chunk's 4 s-tiles
                    for stl in range(4):
                        st = c * 4 + stl
                        ow1, ow2 = ow[0][st], ow[1][st]
                        rec = pG.tile([128, 4], F32, tag="rec", name=f"rec{st}")
                        nc.vector.reciprocal(rec, _rview(ow2))
                        rho = pG.tile([128, 4], F32, tag="rho", name=f"rho{st}")
                        nc.vector.tensor_tensor(out=rho, in0=_rview(ow1), in1=rec, op=MULT)
                        nc.vector.tensor_tensor(out=rho, in0=rho, in1=lam_b, op=MULT)
                        dt_ = pG.tile([128, THDC], F32, tag="dt", name=f"dt{st}")
                        dtv = dt_[:, :].rearrange("p (h c) -> p h c", h=HPC)
                        nc.vector.tensor_tensor(out=dtv, in0=_hview(ow2), in1=_b64(rho), op=MULT)
                        nc.vector.tensor_tensor(out=dtv, in0=_hview(ow1), in1=dtv, op=SUB)
                        s1 = pG.tile([128, 4], F32, tag="s1", name=f"s1{st}")
                        nc.vector.reduce_sum(out=s1, in_=dtv, axis=AX)
                        nc.vector.tensor_scalar_mul(s1, s1, -1.0 / Hd)
                        nc.vector.tensor_tensor(out=dtv, in0=dtv, in1=_b64(s1), op=ADD)
                        d2 = pG.tile([128, THDC], F32, tag="d2", name=f"d2{st}")
                        nc.gpsimd.tensor_tensor(out=d2, in0=dt_, in1=dt_, op=MULT)
                        s2 = pG.tile([128, 4], F32, tag="s2", name=f"s2{st}")
                        nc.vector.reduce_sum(out=s2,
                                             in_=d2[:, :].rearrange("p (h c) -> p h c", h=HPC),
                                             axis=AX)
                        nc.vector.tensor_tensor(out=rec, in0=_rview(ow1), in1=_rview(ow1), op=MULT)
                        nc.vector.tensor_scalar_mul(rec, rec, GN_EPS)
                        nc.vector.tensor_scalar_mul(s2, s2, 1.0 / Hd)
                        nc.vector.tensor_tensor(out=s2, in0=s2, in1=rec, op=ADD)
                        nc.scalar.activation(out=s2, in_=s2, func=AF.Sqrt)
                        nc.vector.reciprocal(s2, s2)
                        nc.vector.tensor_tensor(out=dtv, in0=dtv, in1=_b64(s2), op=MULT)
                        nc.gpsimd.tensor_tensor(out=dt_, in0=dt_, in1=gnw_eff, op=MULT)
                        nc.gpsimd.tensor_tensor(out=dt_, in0=dt_, in1=gnb_eff, op=ADD)
                        for ci in range(2):
                            ptr = pPt2.tile([128, 128], F32, tag="ptr2", name=f"py{st}_{ci}")
                            nc.tensor.transpose(ptr, dt_[:, ci * 128:(ci + 1) * 128], ident)
                            nc.vector.tensor_copy(out=yt[ci][:, st * 128:(st + 1) * 128],
                                                  in_=ptr)
                        for oc in range(2):
                            pout = pPout.tile([128, 512], F32, tag="pout", name=f"pout{st}_{oc}")
                            for ci in range(2):
                                nc.tensor.matmul(pout,
                                                 lhsT=yt[ci][:, st * 128:(st + 1) * 128],
                                                 rhs=wo_sb[ci][:, oc * 512:(oc + 1) * 512],
                                                 start=(ci == 0), stop=(ci == 1))
                            ost = pOut.tile([128, 512], F32, tag="ost", name=f"ost{st}_{oc}")
                            nc.vector.tensor_copy(out=ost, in_=pout)
                            dma(out=y_d[st * 128:(st + 1) * 128, oc * 512:(oc + 1) * 512],
                                in_=ost)

    _split_multi_waits(nc)
    return nc


_CACHE = {}


def _get_module():
    if "nc" not in _CACHE:
        _CACHE["nc"] = build_module()
        _CACHE["tables"] = _rope_tables()
    return _CACHE["nc"], _CACHE["tables"]


def kernel(x, Wq, Wk, Wv, Wo, lambda_q1, lambda_k1, lambda_q2, lambda_k2,
           lambda_init, gn_weight, gn_bias):
    from concourse.bass_utils import run_bass_kernel_spmd

    x = np.ascontiguousarray(np.asarray(x, dtype=np.float32))
    Wq = np.asarray(Wq, dtype=np.float32)
    Wk = np.asarray(Wk, dtype=np.float32)
    Wv = np.asarray(Wv, dtype=np.float32)
    Wo = np.asarray(Wo, dtype=np.float32)
    lq1 = np.asarray(lambda_q1, dtype=np.float32)
    lk1 = np.asarray(lambda_k1, dtype=np.float32)
    lq2 = np.asarray(lambda_q2, dtype=np.float32)
    lk2 = np.asarray(lambda_k2, dtype=np.float32)
    lam_init = np.float32(np.asarray(lambda_init).reshape(()))
    gnw = np.asarray(gn_weight, dtype=np.float32)
    gnb = np.asarray(gn_bias, dtype=np.float32)

    nc, (cosT, ssinT) = _get_module()

    in_maps = []
    for core in range(8):
        b = core // 4
        hb = (core % 4) * HPC
        c1 = slice(hb * Hd, (hb + HPC) * Hd)
        c2 = slice(H * Hd + hb * Hd, H * Hd + (hb + HPC) * Hd)
        lam = np.stack([lq1[hb:hb + HPC], lk1[hb:hb + HPC],
                        lq2[hb:hb + HPC], lk2[hb:hb + HPC],
                        np.full(HPC, lam_init, np.float32)]).astype(np.float32)
        in_maps.append({
            "x": np.ascontiguousarray(x[b]),
            "wq": np.ascontiguousarray(np.concatenate([Wq[:, c1], Wq[:, c2]], axis=1)),
            "wk": np.ascontiguousarray(np.concatenate([Wk[:, c1], Wk[:, c2]], axis=1)),
            "wv": np.ascontiguousarray(Wv[:, c1]),
            "wo": np.ascontiguousarray(Wo[c1, :]),
            "lam": lam,
            "gnw": np.ascontiguousarray(gnw[c1]),
            "gnb": np.ascontiguousarray(gnb[c1]),
            "cosT": cosT,
            "ssinT": ssinT,
        })

    last_err = None
    for attempt in range(3):
        try:
            res = run_bass_kernel_spmd(nc, in_maps, core_ids=list(range(8)))
            break
        except Exception as e:  # transient axon/device hiccups
            last_err = e
            time.sleep(10 * (attempt + 1))
    else:
        raise last_err

    out = np.zeros((B, S, D), dtype=np.float32)
    for core in range(8):
        out[core // 4] += res.results[core]["y"]
    return out
